# revision 46
# baseline (speedup 1.0000x reference)
"""Trainium2 Bass kernel for a GPT-2 style transformer block.

Problem: B=4, T=2048, C=1024, H=16 heads (hd=64), MLP hidden 4096, fp32 I/O.

Sharding: zero-collective 8-way data parallel with a *folded* sequence
split. Core c handles batch b=c//2; the pair of cores for a batch element
split the 2048 query tokens into eight 256-token chunks, interleaved so
causal attention work is balanced AND the union tile-set is SPMD-uniform:
  core s=0: chunks {0, 3, 4, 7}   core s=1: chunks {1, 2, 5, 6}
Attention runs in 4 slots; slot g covers the core's g-th chunk (ascending
need) against the natural-order key prefix of K_g = 4(g+1) key tiles of
128. The last 4 key tiles of each slot carry per-core element masks
(ones / partial-diagonal / zero) from DRAM; earlier tiles are full for
both cores. No gating, no wasted "other half" tiles.

K/V are computed for the full 2048 natural-order tokens (x_seq); Q and the
residual stream use a second input x_q holding the core's own 1024 tokens
in slot order (host-side gather).

exp() is split between the Scalar engine (true exp) and the Vector engine
(Schraudolph int-bitcast exp2 approximation) so neither is a bottleneck.

fp8 (e4m3, DoubleRow) everywhere error budget allows: QKV projections,
MLP c_fc and c_proj. Attention scores/AV and attn.c_proj stay bf16.
Weights are host-prescaled into fp8 range; the inverse scale rides the
existing bias-add / activation instructions.
"""

import os
import sys
import types
import math

import numpy as np
import ml_dtypes

for _p in ("/opt/trn_rl_repo", "/root/.axon_site/_ro/trn_rl_repo"):
    if os.path.isdir(_p) and _p not in sys.path:
        sys.path.append(_p)

if "antenv.axon_hooks" not in sys.modules:
    try:
        import antenv

        _m = types.ModuleType("antenv.axon_hooks")
        _m._hook = None

        def _set_hook(h):
            _m._hook = h

        def _get_hook():
            return _m._hook

        _m.set_axon_ntff_profile_hook = _set_hook
        _m.get_axon_ntff_profile_hook = _get_hook
        sys.modules["antenv.axon_hooks"] = _m
        antenv.axon_hooks = _m
    except ImportError:
        pass

import concourse.bacc as bacc
import concourse.tile as tile
from concourse import mybir
from concourse.masks import make_identity

P = 128
B, T, C = 4, 2048, 1024
H, HD = 16, 64
F = 4096
T2 = T
TQ = T // 2
CC = C // P
N = 512
QW = 256  # query chunk width

CHUNKS = {0: [0, 3, 4, 7], 1: [1, 2, 5, 6]}

F32 = mybir.dt.float32
F32R = mybir.dt.float32r
BF16 = mybir.dt.bfloat16
F8 = mybir.dt.float8e4
I32 = mybir.dt.int32
AF = mybir.ActivationFunctionType
OP = mybir.AluOpType
DR = mybir.MatmulPerfMode.DoubleRow

FC_SCALE = 32.0
OT_SCALE = 256.0
# Schraudolph exp: i = A*s + B; bitcast i (int32) as float32 ~= exp(0.125*s)
SCHR_A = (1 << 23) * (1.0 / math.log(2.0)) * 0.125
SCHR_B = (127.0 - 0.0434609) * (1 << 23) + 0.5

PROFILE = False
last_exec_time_ns = None

_CACHE = {}


def _exp_on_act(h):
    return True


def _build_nc(apply_lnwb: bool = True, apply_bias: bool = True):
    nc = bacc.Bacc("TRN2", target_bir_lowering=False, debug=False, num_devices=8)

    x_seq = nc.dram_tensor("x_seq", [T2, C], F32, kind="ExternalInput")
    x_q = nc.dram_tensor("x_q", [TQ, C], F32, kind="ExternalInput")
    w_qk = nc.dram_tensor("w_qk", [512, 2 * 2 * C], F8, kind="ExternalInput")
    w_v = nc.dram_tensor("w_v", [512, 2 * C], F8, kind="ExternalInput")
    w_pj = nc.dram_tensor("w_pj", [C, C], BF16, kind="ExternalInput")
    w_fc = nc.dram_tensor("w_fc", [512, 8192], F8, kind="ExternalInput")
    w_ot = nc.dram_tensor("w_ot", [2048, 2048], F8, kind="ExternalInput")
    ln1w = nc.dram_tensor("ln1w", [P, C], BF16, kind="ExternalInput")
    ln1b = nc.dram_tensor("ln1b", [P, C], BF16, kind="ExternalInput")
    ln2w = nc.dram_tensor("ln2w", [P, C], BF16, kind="ExternalInput")
    ln2b = nc.dram_tensor("ln2b", [P, C], BF16, kind="ExternalInput")
    b_q = nc.dram_tensor("b_q", [P, CC], F32, kind="ExternalInput")
    b_k = nc.dram_tensor("b_k", [P, CC], F32, kind="ExternalInput")
    b_v = nc.dram_tensor("b_v", [P, C], F32, kind="ExternalInput")
    b_pj = nc.dram_tensor("b_pj", [P, C], F32, kind="ExternalInput")
    b_fc = nc.dram_tensor("b_fc", [P, F // P], F32, kind="ExternalInput")
    b_ot = nc.dram_tensor("b_ot", [P, C], F32, kind="ExternalInput")
    masks = nc.dram_tensor("masks", [P, 16 * QW], BF16, kind="ExternalInput")
    sel2 = nc.dram_tensor("sel2", [2, P], F32R, kind="ExternalInput")

    out_d = nc.dram_tensor("out", [TQ, C], F32, kind="ExternalOutput")

    from contextlib import ExitStack

    with tile.TileContext(nc) as tc, ExitStack() as ctx:
        const = ctx.enter_context(tc.tile_pool(name="const", bufs=1))
        es_mid = ctx.enter_context(ExitStack())
        es_h = ctx.enter_context(ExitStack())
        es_yb = ctx.enter_context(ExitStack())
        es_qkv = ctx.enter_context(ExitStack())
        es_xln = ctx.enter_context(ExitStack())
        es_pj = ctx.enter_context(ExitStack())
        p_mid = es_mid.enter_context(tc.tile_pool(name="p_mid", bufs=1))
        p_qkv = es_qkv.enter_context(tc.tile_pool(name="p_qkv", bufs=1))
        p_xln = es_xln.enter_context(tc.tile_pool(name="p_xln", bufs=1))

        ident = const.tile([P, P], BF16, tag="ident", name="ident")
        make_identity(nc, ident)
        eps_sb = const.tile([P, 1], F32, tag="eps", name="eps")
        nc.vector.memset(eps_sb[:], 1e-5)
        # warm-up filler: keep the PE busy for the HAM activity window right
        # from t=0 so the first real matmuls run at 2.4GHz instead of 1.2
        warm_src = const.tile([P, P], BF16, tag="warm_src", name="warm_src")
        nc.vector.memset(warm_src[:], 0.5)
        with tc.tile_pool(name="warm_ps", bufs=2, space="PSUM") as wps:
            for _ in range(24):
                wpt = wps.tile([P, P], BF16, tag="warm", name="warm")
                nc.tensor.transpose(wpt[:], warm_src[:], ident[:])
        sel_sb = const.tile([2, P], F32R, tag="sel", name="sel")
        nc.sync.dma_start(out=sel_sb[:], in_=sel2[:])
        bq_sb = const.tile([P, CC], F32, tag="bq", name="bq")
        nc.sync.dma_start(out=bq_sb[:], in_=b_q[:])
        bk_sb = const.tile([P, CC], F32, tag="bk", name="bk")
        nc.sync.dma_start(out=bk_sb[:], in_=b_k[:])
        bfc_sb = const.tile([P, F // P], F32, tag="bfc", name="bfc")
        nc.sync.dma_start(out=bfc_sb[:], in_=b_fc[:])

        q_sb = [p_qkv.tile([P, TQ], BF16, tag=f"q{i}", name=f"q{i}") for i in range(CC)]
        k_sb = [p_qkv.tile([P, T2], BF16, tag=f"k{i}", name=f"k{i}") for i in range(CC)]
        v_sb = [p_qkv.tile([P, H * 65], BF16, tag=f"v{i}", name=f"v{i}") for i in range(T2 // P)]
        xlnT_big = p_xln.tile([P, CC, T2], F8, tag="xlnT", name="xlnT")
        xlnTq_big = p_xln.tile([P, CC, TQ], F8, tag="xlnTq", name="xlnTq")

        # ---------------- Phase 1: LN1 + transposes + QKV (fp8 DoubleRow) --
        # x_seq (natural order, 16 tiles) feeds K and V; x_q (own tokens in
        # slot order, 8 tiles) feeds Q. V per tile keeps the PE busy during
        # the DVE-bound LN pipeline; K/Q blocks fire every 4th tile.
        with tc.tile_pool(name="ln1_sp", bufs=3) as sp, \
             tc.tile_pool(name="ln1_cp", bufs=1) as cp, \
             tc.tile_pool(name="ln1_st", bufs=6) as st, \
             tc.tile_pool(name="qkv_w", bufs=4) as wp, \
             tc.tile_pool(name="v_w", bufs=4) as vwp, \
             tc.tile_pool(name="qkv_ps", bufs=3, space="PSUM") as qps, \
             tc.tile_pool(name="v_ps", bufs=2, space="PSUM") as vps, \
             tc.tile_pool(name="ln1_ps", bufs=2, space="PSUM") as tps:
            wqk_sb = []
            wv_sb = []
            for g in range(4):
                wt = vwp.tile([P, 2, C], F8, tag="wv8", name="wv8")
                nc.sync.dma_start(out=wt[:],
                                  in_=w_v[g * P:(g + 1) * P, :].rearrange(
                                      "p (j n) -> p j n", j=2))
                wv_sb.append(wt)
            for g in range(4):
                wt = wp.tile([P, 2, 2 * C], F8, tag="w8", name="w8")
                nc.sync.dma_start(out=wt[:],
                                  in_=w_qk[g * P:(g + 1) * P, :].rearrange(
                                      "p (j n) -> p j n", j=2))
                wqk_sb.append(wt)
            bv_sb = cp.tile([P, C], F32, tag="bv", name="bv")
            nc.sync.dma_start(out=bv_sb[:], in_=b_v[:])
            for kt in range(T2 // P):
                nc.gpsimd.memset(
                    v_sb[kt].rearrange("p (h d) -> p h d", d=65)[:, :, 64:65], 1.0)
            if apply_lnwb:
                w1 = cp.tile([P, C], BF16, tag="w1", name="w1")
                nc.sync.dma_start(out=w1[:], in_=ln1w[:])
                b1 = cp.tile([P, C], BF16, tag="b1", name="b1")
                nc.sync.dma_start(out=b1[:], in_=ln1b[:])
            # interleave x_q tiles early so Q blk0/blk1 (and with them the
            # first attention slots) are ready mid-phase
            ti_order = ([0, 1, 2, 3] + [16, 17, 18, 19] + [4, 5, 6, 7]
                        + [20, 21, 22, 23] + list(range(8, 16)))
            for ti in ti_order:
                is_seq = ti < 16
                tt = ti if is_seq else ti - 16
                src = x_seq if is_seq else x_q
                dstT = xlnT_big if is_seq else xlnTq_big
                xt = sp.tile([P, C], F32, tag="xs", name="xs")
                nc.sync.dma_start(out=xt[:], in_=src[tt * P:(tt + 1) * P, :])
                stats = st.tile([P, 2, 6], F32, tag="st", name="st")
                for g in range(2):
                    nc.vector.bn_stats(out=stats[:, g, :], in_=xt[:, g * 512:(g + 1) * 512])
                mv = st.tile([P, 2], F32, tag="mv", name="mv")
                nc.vector.bn_aggr(out=mv[:], in_=stats[:])
                rstd = st.tile([P, 1], F32, tag="rstd", name="rstd")
                nc.scalar.activation(rstd[:], mv[:, 1:2], AF.Sqrt, bias=eps_sb[:], scale=1.0)
                nc.vector.reciprocal(out=rstd[:], in_=rstd[:])
                xb = sp.tile([P, C], BF16, tag="xb", name="xb")
                nmr = st.tile([P, 1], F32, tag="nmr", name="nmr")
                nc.vector.tensor_scalar(nmr[:], mv[:, 0:1], rstd[:], -1.0,
                                        OP.mult, OP.mult)
                if apply_lnwb:
                    xc = sp.tile([P, C], F32, tag="xc", name="xc")
                    nc.scalar.activation(xc[:], xt[:], AF.Identity,
                                         bias=nmr[:], scale=rstd[:])
                    nc.vector.tensor_tensor(out=xc[:], in0=xc[:], in1=w1[:], op=OP.mult)
                    nc.vector.tensor_tensor(out=xb[:], in0=xc[:], in1=b1[:], op=OP.add)
                else:
                    nc.scalar.activation(xb[:], xt[:], AF.Identity,
                                         bias=nmr[:], scale=rstd[:])
                pst = tps.tile([P, CC, P], BF16, tag="trp", name="trp")
                for cc in range(CC):
                    nc.tensor.transpose(pst[:, cc, :],
                                        xb[:, cc * P:(cc + 1) * P], ident[:])
                # alternate the psum->SBUF f8 convert between DVE and ACT so
                # neither paces the LN pipeline
                if ti % 2 == 0:
                    nc.vector.tensor_copy(
                        out=dstT[:, :, tt * P:(tt + 1) * P], in_=pst[:])
                else:
                    nc.scalar.activation(
                        dstT[:, :, tt * P:(tt + 1) * P], pst[:],
                        AF.Identity, bias=0.0, scale=1.0)
                if is_seq:
                    # V for this token tile
                    for vg in range(2):
                        ps = vps.tile([P, N], F32, tag="v_ps", name="v_ps")
                        for g in range(4):
                            nc.tensor.matmul(ps[:],
                                             lhsT=xlnT_big[:, 2 * g:2 * g + 2,
                                                           tt * P:(tt + 1) * P],
                                             rhs=wv_sb[g][:, :, vg * N:(vg + 1) * N],
                                             start=(g == 0), stop=(g == 3),
                                             perf_mode=DR)
                        out_ap = v_sb[tt].rearrange("p (h d) -> p h d", d=65)[
                            :, vg * 8:(vg + 1) * 8, 0:64]
                        in_ap = ps.rearrange("p (h d) -> p h d", d=64)[:, :, :]
                        bv_ap = bv_sb.rearrange("p (h d) -> p h d", d=64)[
                            :, vg * 8:(vg + 1) * 8, :]
                        nc.vector.scalar_tensor_tensor(
                            out=out_ap, in0=in_ap, scalar=1.0 / FC_SCALE,
                            in1=bv_ap, op0=OP.mult, op1=OP.add)
                if tt % 4 == 3:
                    blk = tt // 4
                    for fc in range(CC):
                        ps = qps.tile([P, N], F32, tag="qk_ps", name="qk_ps")
                        srcT = xlnT_big if is_seq else xlnTq_big
                        fco = CC + fc if is_seq else fc
                        for g in range(4):
                            nc.tensor.matmul(ps[:],
                                             lhsT=wqk_sb[g][:, :, fco * P:(fco + 1) * P],
                                             rhs=srcT[:, 2 * g:2 * g + 2,
                                                      blk * N:(blk + 1) * N],
                                             start=(g == 0), stop=(g == 3),
                                             perf_mode=DR)
                        if is_seq:
                            nc.scalar.activation(
                                k_sb[fc][:, blk * N:(blk + 1) * N], ps[:],
                                AF.Identity, bias=bk_sb[:, fc:fc + 1],
                                scale=1.0 / FC_SCALE)
                        else:
                            nc.scalar.activation(
                                q_sb[fc][:, blk * N:(blk + 1) * N], ps[:],
                                AF.Identity, bias=bq_sb[:, fc:fc + 1],
                                scale=1.0 / FC_SCALE)

        es_xln.close()

        x_mid = [p_mid.tile([P, C], F32, tag=f"xm{i}", name=f"xm{i}") for i in range(TQ // P)]
        p_yb = es_yb.enter_context(tc.tile_pool(name="p_yb", bufs=1))
        y_fm = [p_yb.tile([P, TQ], BF16, tag=f"yf{i}", name=f"yf{i}") for i in range(CC)]
        s16 = p_yb.tile([16, TQ], F32, tag="s16", name="s16")
        xb2_t = [p_yb.tile([P, C], BF16, tag=f"xb2_{i}", name=f"xb2_{i}")
                 for i in range(TQ // P)]
        mask_sb = p_yb.tile([P, 16, QW], BF16, tag="mask", name="mask")
        nc.sync.dma_start(out=mask_sb[:],
                          in_=masks[:].rearrange("p (m q) -> p m q", q=QW))
        p_pjw = es_pj.enter_context(tc.tile_pool(name="pj_w", bufs=1))
        wpj_sb = [p_pjw.tile([P, C], BF16, tag=f"wpj{i}", name=f"wpj{i}") for i in range(CC)]
        for i in range(CC):
            nc.sync.dma_start(out=wpj_sb[i][:], in_=w_pj[i * P:(i + 1) * P, :])
        if apply_bias:
            bpj_sb = p_pjw.tile([P, C], F32, tag="bpj", name="bpj")
            nc.sync.dma_start(out=bpj_sb[:], in_=b_pj[:])
        if apply_lnwb:
            w2 = p_pjw.tile([P, C], BF16, tag="w2", name="w2")
            nc.sync.dma_start(out=w2[:], in_=ln2w[:])
            b2 = p_pjw.tile([P, C], BF16, tag="b2", name="b2")
            nc.sync.dma_start(out=b2[:], in_=ln2b[:])

        # ---------------- Phase 2: attention slots + proj, pipelined -------
        with tc.tile_pool(name="att_sp", bufs=3) as ap_pool, \
             tc.tile_pool(name="att_s1", bufs=2) as s1_pool, \
             tc.tile_pool(name="pj_sp", bufs=2) as pj_sp, \
             tc.tile_pool(name="att_sps", bufs=2, space="PSUM") as sps_pool, \
             tc.tile_pool(name="att_yps", bufs=2, space="PSUM") as yps_pool, \
             tc.tile_pool(name="att_rps", bufs=1, space="PSUM") as rps_pool, \
             tc.tile_pool(name="pj_ps", bufs=1, space="PSUM") as pps:
            for slot in range(4):
                n_sg = slot + 1
                qlo = slot * QW
                for h in range(H):
                    hp, ro = h // 2, (h % 2) * 64
                    yps = yps_pool.tile([65, QW], F32, tag="yps", name="yps")
                    for sg in range(n_sg):
                        sp4 = sps_pool.tile([P, 4, QW], F32, tag="sps", name="sps")
                        for j in range(4):
                            kt = sg * 4 + j
                            nc.tensor.matmul(sp4[:, j, :],
                                             lhsT=k_sb[hp][ro:ro + 64, kt * P:(kt + 1) * P],
                                             rhs=q_sb[hp][ro:ro + 64, qlo:qlo + QW],
                                             start=True, stop=True)
                        maskable = (sg == n_sg - 1)
                        at4 = ap_pool.tile([P, 4, QW], BF16, tag="at", name="at")
                        nc.scalar.activation(at4[:], sp4[:], AF.Exp,
                                             bias=0.0, scale=0.125)
                        if maskable:
                            nc.vector.tensor_tensor(
                                out=at4[:], in0=at4[:],
                                in1=mask_sb[:, 4 * slot:4 * slot + 4, :],
                                op=OP.mult)
                        for j in range(4):
                            kt = sg * 4 + j
                            nc.tensor.matmul(yps[:],
                                             lhsT=v_sb[kt][:, h * 65:(h + 1) * 65],
                                             rhs=at4[:, j, :],
                                             start=(sg == 0 and j == 0),
                                             stop=(sg == n_sg - 1 and j == 3))
                    # stash y and the softmax row-sums; normalize per-slot below
                    nc.vector.tensor_copy(out=y_fm[hp][ro:ro + 64, qlo:qlo + QW],
                                          in_=yps[0:64, :])
                    s1 = s1_pool.tile([1, QW], F32, tag="s1", name="s1")
                    nc.vector.tensor_copy(out=s1[:], in_=yps[64:65, :])
                    nc.sync.dma_start(out=s16[h:h + 1, qlo:qlo + QW], in_=s1[:])
                # normalize all heads for this slot: one batched reciprocal,
                # per-pair repack via SBUF->SBUF DMA, K=2 selector matmul
                # broadcast, multiply on gpsimd
                recip = s1_pool.tile([16, QW], F32, tag="recip", name="recip")
                nc.vector.reciprocal(out=recip[:], in_=s16[:, qlo:qlo + QW])
                recipr = s1_pool.tile([16, QW], F32R, tag="recipr", name="recipr")
                nc.vector.tensor_copy(out=recipr[:], in_=recip[:])
                for yt in range(CC):
                    rr2 = s1_pool.tile([2, QW], F32R, tag="rr2", name="rr2")
                    nc.sync.dma_start(out=rr2[:], in_=recipr[2 * yt:2 * yt + 2, :])
                    rps = rps_pool.tile([P, QW], F32, tag="rps", name="rps")
                    nc.tensor.matmul(rps[:], lhsT=sel_sb[:], rhs=rr2[:],
                                     start=True, stop=True)
                    nc.vector.tensor_tensor(out=y_fm[yt][:, qlo:qlo + QW],
                                            in0=y_fm[yt][:, qlo:qlo + QW],
                                            in1=rps[:], op=OP.mult)
                # proj + residual + LN2 for this slot's two 128-token tiles
                for t8 in (2 * slot, 2 * slot + 1):
                    xo = pj_sp.tile([P, C], F32, tag="xo", name="xo")
                    nc.sync.dma_start(out=xo[:], in_=x_q[t8 * P:(t8 + 1) * P, :])
                    for ft in range(2):
                        ps = pps.tile([P, N], F32, tag="pj_ps", name="pj_ps")
                        for yc in range(CC):
                            nc.tensor.matmul(ps[:],
                                             lhsT=y_fm[yc][:, t8 * P:(t8 + 1) * P],
                                             rhs=wpj_sb[yc][:, ft * N:(ft + 1) * N],
                                             start=(yc == 0), stop=(yc == CC - 1))
                        nc.vector.tensor_tensor(out=x_mid[t8][:, ft * N:(ft + 1) * N],
                                                in0=ps[:], in1=xo[:, ft * N:(ft + 1) * N],
                                                op=OP.add)
                    if apply_bias:
                        nc.vector.tensor_tensor(out=x_mid[t8][:], in0=x_mid[t8][:],
                                                in1=bpj_sb[:], op=OP.add)
                    # LN2 stats + normalized activations (SBUF only); the
                    # feature-major transposes run after the slot loop
                    xt = x_mid[t8]
                    stats = s1_pool.tile([P, 2, 6], F32, tag="st2", name="st2")
                    for g in range(2):
                        nc.vector.bn_stats(out=stats[:, g, :], in_=xt[:, g * 512:(g + 1) * 512])
                    mv = s1_pool.tile([P, 2], F32, tag="mv2", name="mv2")
                    nc.vector.bn_aggr(out=mv[:], in_=stats[:])
                    rstd = s1_pool.tile([P, 1], F32, tag="rstd2", name="rstd2")
                    nc.scalar.activation(rstd[:], mv[:, 1:2], AF.Sqrt, bias=eps_sb[:], scale=1.0)
                    nc.vector.reciprocal(out=rstd[:], in_=rstd[:])
                    nmr = s1_pool.tile([P, 1], F32, tag="nmr2", name="nmr2")
                    nc.vector.tensor_scalar(nmr[:], mv[:, 0:1], rstd[:], -1.0,
                                            OP.mult, OP.mult)
                    if apply_lnwb:
                        xc2 = pj_sp.tile([P, C], BF16, tag="xc2", name="xc2")
                        nc.scalar.activation(xc2[:], xt[:], AF.Identity,
                                             bias=nmr[:], scale=rstd[:])
                        nc.vector.tensor_tensor(out=xc2[:], in0=xc2[:], in1=w2[:], op=OP.mult)
                        nc.vector.tensor_tensor(out=xb2_t[t8][:], in0=xc2[:], in1=b2[:], op=OP.add)
                    else:
                        nc.scalar.activation(xb2_t[t8][:], xt[:], AF.Identity,
                                             bias=nmr[:], scale=rstd[:])

        es_pj.close()
        es_yb.close()
        es_qkv.close()

        # ---------------- Phase 3: LN2 transposes (stats done in-slot) -----
        p_h = es_h.enter_context(tc.tile_pool(name="p_h", bufs=1))
        h_big = p_h.tile([P, F // P, TQ], F8, tag="hbig", name="hbig")
        xln2T_big = p_h.tile([P, CC, TQ], F8, tag="xln2T", name="xln2T")
        with tc.tile_pool(name="ln2_ps", bufs=2, space="PSUM") as tps:
            for tt in range(TQ // P):
                pst = tps.tile([P, CC, P], BF16, tag="trp2", name="trp2")
                for cc in range(CC):
                    nc.tensor.transpose(pst[:, cc, :],
                                        xb2_t[tt][:, cc * P:(cc + 1) * P], ident[:])
                if tt % 2 == 0:
                    nc.vector.tensor_copy(
                        out=xln2T_big[:, :, tt * P:(tt + 1) * P], in_=pst[:])
                else:
                    nc.scalar.activation(
                        xln2T_big[:, :, tt * P:(tt + 1) * P], pst[:],
                        AF.Identity, bias=0.0, scale=1.0)

        # ---------------- Phase 4: FC + gelu (fp8 DoubleRow) ---------------
        with tc.tile_pool(name="fc_w", bufs=12) as wp, \
             tc.tile_pool(name="fc_ps", bufs=4, space="PSUM") as fps:
            for hg in range(F // N):
                wts = []
                for g in range(4):
                    wt = wp.tile([P, 2, N], F8, tag="wfc", name="wfc")
                    nc.sync.dma_start(
                        out=wt[:],
                        in_=w_fc[g * P:(g + 1) * P,
                                 hg * 2 * N:(hg + 1) * 2 * N].rearrange(
                                     "p (j n) -> p j n", j=2))
                    wts.append(wt)
                for hs in range(4):
                    hf = hg * 4 + hs
                    for tt in range(2):
                        ps = fps.tile([P, N], F32, tag="fc_ps", name="fc_ps")
                        for g in range(4):
                            nc.tensor.matmul(ps[:],
                                             lhsT=wts[g][:, :, hs * P:(hs + 1) * P],
                                             rhs=xln2T_big[:, 2 * g:2 * g + 2,
                                                           tt * N:(tt + 1) * N],
                                             start=(g == 0), stop=(g == 3),
                                             perf_mode=DR)
                        nc.scalar.activation(h_big[:, hf, tt * N:(tt + 1) * N], ps[:],
                                             AF.Gelu_apprx_tanh,
                                             bias=bfc_sb[:, hf:hf + 1],
                                             scale=1.0 / FC_SCALE)

        # ---------------- Phase 5: out matmul + residual (fp8 DoubleRow) ---
        with tc.tile_pool(name="ot_w", bufs=4) as wp, \
             tc.tile_pool(name="ot_cp", bufs=1) as cp, \
             tc.tile_pool(name="ot_sp", bufs=3) as sp, \
             tc.tile_pool(name="ot_ps", bufs=8, space="PSUM") as ops_pool:
            if apply_bias:
                bot_sb = cp.tile([P, C], F32, tag="bot", name="bot")
                nc.sync.dma_start(out=bot_sb[:], in_=b_ot[:])
            for half in range(2):
                opss = [ops_pool.tile([P, N], F32, tag="ot_ps", name="ot_ps") for _ in range(8)]
                for q in range(F // (2 * P)):
                    wt = wp.tile([P, 2, C], F8, tag="wot", name="wot")
                    nc.sync.dma_start(
                        out=wt[:],
                        in_=w_ot[q * P:(q + 1) * P, :].rearrange(
                            "p (j n) -> p j n", j=2))
                    for tc4 in range(4):
                        t8 = half * 4 + tc4
                        for ft in range(2):
                            nc.tensor.matmul(opss[tc4 * 2 + ft][:],
                                             lhsT=h_big[:, 2 * q:2 * q + 2,
                                                        t8 * P:(t8 + 1) * P],
                                             rhs=wt[:, :, ft * N:(ft + 1) * N],
                                             start=(q == 0),
                                             stop=(q == F // (2 * P) - 1),
                                             perf_mode=DR)
                for tc4 in range(4):
                    t8 = half * 4 + tc4
                    ot = sp.tile([P, C], F32, tag="ot", name="ot")
                    for ft in range(2):
                        nc.vector.scalar_tensor_tensor(
                            out=ot[:, ft * N:(ft + 1) * N],
                            in0=opss[tc4 * 2 + ft][:],
                            scalar=1.0 / OT_SCALE,
                            in1=x_mid[t8][:, ft * N:(ft + 1) * N],
                            op0=OP.mult, op1=OP.add)
                    if apply_bias:
                        nc.vector.tensor_tensor(out=ot[:], in0=ot[:], in1=bot_sb[:], op=OP.add)
                    nc.sync.dma_start(out=out_d[t8 * P:(t8 + 1) * P, :], in_=ot[:])

    nc.finalize()
    return nc


def _pack_w8(W, scale):
    """Pack W [M, C] into DoubleRow DRAM layout [4*128, 2*M]:
    out[g*128 + k, j*M + m] = W[m, g*256 + j*128 + k] * scale."""
    w = np.clip(np.asarray(W, np.float32) * scale, -240, 240)
    M = w.shape[0]
    a = w.reshape(M, 4, 2, 128).transpose(1, 3, 2, 0)
    return np.ascontiguousarray(a.reshape(512, 2 * M)).astype(ml_dtypes.float8_e4m3)


def _pack_fc8(W_fc):
    w = np.clip(np.asarray(W_fc, np.float32) * FC_SCALE, -240, 240)
    a = w.reshape(8, 512, 4, 2, 128).transpose(2, 4, 0, 3, 1)
    return np.ascontiguousarray(a.reshape(512, 8192)).astype(ml_dtypes.float8_e4m3)


def _pack_ot8(W_out):
    w = np.clip(np.asarray(W_out, np.float32) * OT_SCALE, -240, 240)
    a = w.reshape(1024, 16, 2, 128).transpose(1, 3, 2, 0)
    return np.ascontiguousarray(a.reshape(2048, 2048)).astype(ml_dtypes.float8_e4m3)


def _make_masks(chunks):
    """mask[p, (4g+i)*QW + qf] = (256*chunks[g] + qf) >= (128*(4g+i) + p)"""
    m = np.zeros((P, 16, QW), np.float32)
    pp = np.arange(P)[:, None]
    qf = np.arange(QW)[None, :]
    for g in range(4):
        c = chunks[g]
        for i in range(4):
            kt = 4 * g + i
            m[:, kt, :] = (QW * c + qf) >= (P * kt + pp)
    return np.ascontiguousarray(m.reshape(P, 16 * QW)).astype(ml_dtypes.bfloat16)


def _prep_shared(inputs):
    bf = ml_dtypes.bfloat16
    W_attn = np.asarray(inputs["W_attn"], np.float32)
    shared = {
        "w_qk": _pack_w8(W_attn[: 2 * C], FC_SCALE),
        "w_v": _pack_w8(W_attn[2 * C:], FC_SCALE),
        "w_pj": np.ascontiguousarray(np.asarray(inputs["W_proj"], np.float32).T).astype(bf),
        "w_fc": _pack_fc8(inputs["W_fc"]),
        "w_ot": _pack_ot8(inputs["W_out"]),
        "ln1w": np.ascontiguousarray(np.broadcast_to(np.asarray(inputs["ln1_w"], np.float32), (P, C))).astype(bf),
        "ln1b": np.ascontiguousarray(np.broadcast_to(np.asarray(inputs["ln1_b"], np.float32), (P, C))).astype(bf),
        "ln2w": np.ascontiguousarray(np.broadcast_to(np.asarray(inputs["ln2_w"], np.float32), (P, C))).astype(bf),
        "ln2b": np.ascontiguousarray(np.broadcast_to(np.asarray(inputs["ln2_b"], np.float32), (P, C))).astype(bf),
        "b_q": np.ascontiguousarray(np.asarray(inputs["b_attn"], np.float32)[:C].reshape(CC, P).T),
        "b_k": np.ascontiguousarray(np.asarray(inputs["b_attn"], np.float32)[C:2 * C].reshape(CC, P).T),
        "b_v": np.ascontiguousarray(np.broadcast_to(np.asarray(inputs["b_attn"], np.float32)[2 * C:], (P, C))),
        "b_pj": np.ascontiguousarray(np.broadcast_to(np.asarray(inputs["b_proj"], np.float32), (P, C))),
        "b_fc": np.ascontiguousarray(np.asarray(inputs["b_fc"], np.float32).reshape(F // P, P).T),
        "b_ot": np.ascontiguousarray(np.broadcast_to(np.asarray(inputs["b_out"], np.float32), (P, C))),
    }
    sel = np.zeros((2, P), np.float32)
    sel[0, :64] = 1.0
    sel[1, 64:] = 1.0
    shared["sel2"] = sel
    return shared


def _make_in_maps(inputs):
    x = np.asarray(inputs["x"], np.float32)
    shared = _prep_shared(inputs)
    masks = {s: _make_masks(CHUNKS[s]) for s in (0, 1)}
    in_maps = []
    for c in range(8):
        b, s = c // 2, c % 2
        m = dict(shared)
        m["x_seq"] = np.ascontiguousarray(x[b])
        m["x_q"] = np.ascontiguousarray(
            np.concatenate([x[b, ch * QW:(ch + 1) * QW] for ch in CHUNKS[s]], axis=0))
        m["masks"] = masks[s]
        in_maps.append(m)
    return in_maps


def _get_nc(apply_lnwb=True, apply_bias=True):
    key = ("nc", apply_lnwb, apply_bias)
    if key not in _CACHE:
        _CACHE[key] = _build_nc(apply_lnwb, apply_bias)
    return _CACHE[key]


def run_cores(inputs, profile=False):
    global last_exec_time_ns
    apply_lnwb = not (
        np.allclose(np.asarray(inputs["ln1_w"]), 1.0)
        and np.allclose(np.asarray(inputs["ln1_b"]), 0.0)
        and np.allclose(np.asarray(inputs["ln2_w"]), 1.0)
        and np.allclose(np.asarray(inputs["ln2_b"]), 0.0))
    apply_bias = not all(
        np.allclose(np.asarray(inputs[k]), 0.0)
        for k in ("b_attn", "b_proj", "b_fc", "b_out"))
    nc = _get_nc(apply_lnwb, apply_bias)
    in_maps = _make_in_maps(inputs)
    if profile:
        import concourse.bass_utils as bass_utils
        bass_utils.upload_artifacts = lambda tmpdir: "local://" + tmpdir
        try:
            from trn_agent_boot.trn_boot import _ntff_profile_via_ctypes
            import antenv.axon_hooks as hooks
            if hooks.get_axon_ntff_profile_hook() is None:
                hooks.set_axon_ntff_profile_hook(
                    _ntff_profile_via_ctypes("/opt/axon/libaxon_pjrt.so"))
        except Exception:
            pass
        res = bass_utils.run_bass_kernel_spmd(nc, in_maps, list(range(8)), trace=True)
        last_exec_time_ns = res.exec_time_ns
        return res.results
    return _cached_runner(nc)(in_maps)


def _cached_runner(nc):
    key = ("runner", id(nc))
    if key in _CACHE:
        return _CACHE[key]
    import jax
    import numpy as _np
    from jax.sharding import Mesh, PartitionSpec
    from jax.experimental.shard_map import shard_map
    from concourse import bass2jax, mybir as _mybir
    bass2jax.install_neuronx_cc_hook()

    part_name = nc.partition_id_tensor.name if nc.partition_id_tensor else None
    in_names, out_names, out_avals, zero_outs = [], [], [], []
    for alloc in nc.m.functions[0].allocations:
        if not isinstance(alloc, _mybir.MemoryLocationSet):
            continue
        name = alloc.memorylocations[0].name
        if alloc.kind == "ExternalInput":
            if name != part_name:
                in_names.append(name)
        elif alloc.kind == "ExternalOutput":
            out_names.append(name)
            shape = tuple(alloc.tensor_shape)
            dtype = _mybir.dt.np(alloc.dtype)
            out_avals.append(jax.core.ShapedArray(shape, dtype))
            zero_outs.append(_np.zeros(shape, dtype))
    n_params = len(in_names)
    all_names = in_names + out_names
    if part_name is not None:
        all_names = all_names + [part_name]
    donate = tuple(range(n_params, n_params + len(out_names)))

    def _body(*args):
        operands = list(args)
        if part_name is not None:
            operands.append(bass2jax.partition_id_tensor())
        outs = bass2jax._bass_exec_p.bind(
            *operands, out_avals=tuple(out_avals), in_names=tuple(all_names),
            out_names=tuple(out_names), lowering_input_output_aliases=(),
            sim_require_finite=True, sim_require_nnan=True, nc=nc)
        return tuple(outs)

    devices = jax.devices()[:8]
    mesh = Mesh(_np.asarray(devices), ("core",))
    spec = (PartitionSpec("core"),) * (n_params + len(out_names))
    sharded = jax.jit(
        shard_map(_body, mesh=mesh, in_specs=spec,
                  out_specs=(PartitionSpec("core"),) * len(out_names),
                  check_rep=False),
        donate_argnums=donate, keep_unused=True)

    def run(in_maps):
        concat_in = [
            _np.concatenate([_np.asarray(in_maps[c][nm]) for c in range(8)], axis=0)
            for nm in in_names]
        concat_zero = [_np.zeros((8 * z.shape[0], *z.shape[1:]), z.dtype)
                       for z in zero_outs]
        out_arrs = sharded(*concat_in, *concat_zero)
        return [
            {nm: _np.asarray(out_arrs[i]).reshape(8, *out_avals[i].shape)[c]
             for i, nm in enumerate(out_names)}
            for c in range(8)]

    _CACHE[key] = run
    return run


def kernel(**inputs) -> np.ndarray:
    results = run_cores(inputs, profile=PROFILE)
    out = np.empty((B, T, C), np.float32)
    for c in range(8):
        b, s = c // 2, c % 2
        r = results[c]["out"]
        for g, ch in enumerate(CHUNKS[s]):
            out[b, ch * QW:(ch + 1) * QW, :] = r[g * QW:(g + 1) * QW]
    return out


# revision 47
# speedup vs baseline: 1.2183x; 1.2183x over previous
"""Trainium2 Bass kernel for a GPT-2 style transformer block.

Problem: B=4, T=2048, C=1024, H=16 heads (hd=64), MLP hidden 4096, fp32 I/O.

Sharding: zero-collective 8-way data parallel with a *folded* sequence
split. Core c handles batch b=c//2; the pair of cores for a batch element
split the 2048 query tokens into eight 256-token chunks, interleaved so
causal attention work is balanced AND the union tile-set is SPMD-uniform:
  core s=0: chunks {0, 3, 4, 7}   core s=1: chunks {1, 2, 5, 6}
Attention runs in 4 slots; slot g covers the core's g-th chunk (ascending
need) against the natural-order key prefix of K_g = 4(g+1) key tiles of
128. The last 4 key tiles of each slot carry per-core element masks
(ones / partial-diagonal / zero) from DRAM; earlier tiles are full for
both cores. No gating, no wasted "other half" tiles.

K/V are computed for the full 2048 natural-order tokens (x_seq); Q and the
residual stream use a second input x_q holding the core's own 1024 tokens
in slot order (host-side gather).

exp() is split between the Scalar engine (true exp) and the Vector engine
(Schraudolph int-bitcast exp2 approximation) so neither is a bottleneck.

fp8 (e4m3, DoubleRow) everywhere error budget allows: QKV projections,
MLP c_fc and c_proj. Attention scores/AV and attn.c_proj stay bf16.
Weights are host-prescaled into fp8 range; the inverse scale rides the
existing bias-add / activation instructions.
"""

import os
import sys
import types
import math

import numpy as np
import ml_dtypes

for _p in ("/opt/trn_rl_repo", "/root/.axon_site/_ro/trn_rl_repo"):
    if os.path.isdir(_p) and _p not in sys.path:
        sys.path.append(_p)

if "antenv.axon_hooks" not in sys.modules:
    try:
        import antenv

        _m = types.ModuleType("antenv.axon_hooks")
        _m._hook = None

        def _set_hook(h):
            _m._hook = h

        def _get_hook():
            return _m._hook

        _m.set_axon_ntff_profile_hook = _set_hook
        _m.get_axon_ntff_profile_hook = _get_hook
        sys.modules["antenv.axon_hooks"] = _m
        antenv.axon_hooks = _m
    except ImportError:
        pass

import concourse.bacc as bacc
import concourse.tile as tile
from concourse import mybir
from concourse.masks import make_identity

P = 128
B, T, C = 4, 2048, 1024
H, HD = 16, 64
F = 4096
T2 = T
TQ = T // 2
CC = C // P
N = 512
QW = 256  # query chunk width

CHUNKS = {0: [0, 3, 4, 7], 1: [1, 2, 5, 6]}

F32 = mybir.dt.float32
F32R = mybir.dt.float32r
BF16 = mybir.dt.bfloat16
F8 = mybir.dt.float8e4
I32 = mybir.dt.int32
AF = mybir.ActivationFunctionType
OP = mybir.AluOpType
DR = mybir.MatmulPerfMode.DoubleRow

FC_SCALE = 32.0
OT_SCALE = 256.0
# Schraudolph exp: i = A*s + B; bitcast i (int32) as float32 ~= exp(0.125*s)
SCHR_A = (1 << 23) * (1.0 / math.log(2.0)) * 0.125
SCHR_B = (127.0 - 0.0434609) * (1 << 23) + 0.5

PROFILE = False
last_exec_time_ns = None

_CACHE = {}


def _exp_on_act(h):
    return True


def _build_nc(apply_lnwb: bool = True, apply_bias: bool = True):
    nc = bacc.Bacc("TRN2", target_bir_lowering=False, debug=False, num_devices=8)

    x_seq = nc.dram_tensor("x_seq", [T2, C], F32, kind="ExternalInput")
    x_q = nc.dram_tensor("x_q", [TQ, C], F32, kind="ExternalInput")
    w_qk = nc.dram_tensor("w_qk", [512, 2 * 2 * C], F8, kind="ExternalInput")
    w_v = nc.dram_tensor("w_v", [512, 2 * C], F8, kind="ExternalInput")
    w_pj = nc.dram_tensor("w_pj", [C, C], BF16, kind="ExternalInput")
    w_fc = nc.dram_tensor("w_fc", [512, 8192], F8, kind="ExternalInput")
    w_ot = nc.dram_tensor("w_ot", [2048, 2048], F8, kind="ExternalInput")
    ln1w = nc.dram_tensor("ln1w", [P, C], BF16, kind="ExternalInput")
    ln1b = nc.dram_tensor("ln1b", [P, C], BF16, kind="ExternalInput")
    ln2w = nc.dram_tensor("ln2w", [P, C], BF16, kind="ExternalInput")
    ln2b = nc.dram_tensor("ln2b", [P, C], BF16, kind="ExternalInput")
    b_q = nc.dram_tensor("b_q", [P, CC], F32, kind="ExternalInput")
    b_k = nc.dram_tensor("b_k", [P, CC], F32, kind="ExternalInput")
    b_v = nc.dram_tensor("b_v", [P, C], F32, kind="ExternalInput")
    b_pj = nc.dram_tensor("b_pj", [P, C], F32, kind="ExternalInput")
    b_fc = nc.dram_tensor("b_fc", [P, F // P], F32, kind="ExternalInput")
    b_ot = nc.dram_tensor("b_ot", [P, C], F32, kind="ExternalInput")
    masks = nc.dram_tensor("masks", [P, 16 * QW], BF16, kind="ExternalInput")
    sel2 = nc.dram_tensor("sel2", [2, P], F32R, kind="ExternalInput")

    out_d = nc.dram_tensor("out", [TQ, C], F32, kind="ExternalOutput")

    from contextlib import ExitStack

    with tile.TileContext(nc) as tc, ExitStack() as ctx:
        const = ctx.enter_context(tc.tile_pool(name="const", bufs=1))
        es_mid = ctx.enter_context(ExitStack())
        es_h = ctx.enter_context(ExitStack())
        es_yb = ctx.enter_context(ExitStack())
        es_qkv = ctx.enter_context(ExitStack())
        es_xln = ctx.enter_context(ExitStack())
        es_pj = ctx.enter_context(ExitStack())
        p_mid = es_mid.enter_context(tc.tile_pool(name="p_mid", bufs=1))
        p_qkv = es_qkv.enter_context(tc.tile_pool(name="p_qkv", bufs=1))
        p_xln = es_xln.enter_context(tc.tile_pool(name="p_xln", bufs=1))

        ident = const.tile([P, P], BF16, tag="ident", name="ident")
        make_identity(nc, ident)
        eps_sb = const.tile([P, 1], F32, tag="eps", name="eps")
        nc.vector.memset(eps_sb[:], 1e-5)
        sel_sb = const.tile([2, P], F32R, tag="sel", name="sel")
        nc.sync.dma_start(out=sel_sb[:], in_=sel2[:])
        bq_sb = const.tile([P, CC], F32, tag="bq", name="bq")
        nc.sync.dma_start(out=bq_sb[:], in_=b_q[:])
        bk_sb = const.tile([P, CC], F32, tag="bk", name="bk")
        nc.sync.dma_start(out=bk_sb[:], in_=b_k[:])
        bfc_sb = const.tile([P, F // P], F32, tag="bfc", name="bfc")
        nc.sync.dma_start(out=bfc_sb[:], in_=b_fc[:])

        q_sb = [p_qkv.tile([P, TQ], BF16, tag=f"q{i}", name=f"q{i}") for i in range(CC)]
        k_sb = [p_qkv.tile([P, T2], BF16, tag=f"k{i}", name=f"k{i}") for i in range(CC)]
        v_sb = [p_qkv.tile([P, H * 65], BF16, tag=f"v{i}", name=f"v{i}") for i in range(T2 // P)]
        xlnT_big = p_xln.tile([P, CC, T2], F8, tag="xlnT", name="xlnT")
        xlnTq_big = p_xln.tile([P, CC, TQ], F8, tag="xlnTq", name="xlnTq")

        # ---------------- Phase 1: LN1 + transposes + QKV (fp8 DoubleRow) --
        # x_seq (natural order, 16 tiles) feeds K and V; x_q (own tokens in
        # slot order, 8 tiles) feeds Q. V per tile keeps the PE busy during
        # the DVE-bound LN pipeline; K/Q blocks fire every 4th tile.
        with tc.tile_pool(name="ln1_sp", bufs=3) as sp, \
             tc.tile_pool(name="ln1_cp", bufs=1) as cp, \
             tc.tile_pool(name="ln1_st", bufs=6) as st, \
             tc.tile_pool(name="qkv_w", bufs=4) as wp, \
             tc.tile_pool(name="v_w", bufs=4) as vwp, \
             tc.tile_pool(name="qkv_ps", bufs=3, space="PSUM") as qps, \
             tc.tile_pool(name="v_ps", bufs=2, space="PSUM") as vps, \
             tc.tile_pool(name="ln1_ps", bufs=2, space="PSUM") as tps:
            wqk_sb = []
            wv_sb = []
            for g in range(4):
                wt = vwp.tile([P, 2, C], F8, tag="wv8", name="wv8")
                nc.sync.dma_start(out=wt[:],
                                  in_=w_v[g * P:(g + 1) * P, :].rearrange(
                                      "p (j n) -> p j n", j=2))
                wv_sb.append(wt)
            for g in range(4):
                wt = wp.tile([P, 2, 2 * C], F8, tag="w8", name="w8")
                nc.sync.dma_start(out=wt[:],
                                  in_=w_qk[g * P:(g + 1) * P, :].rearrange(
                                      "p (j n) -> p j n", j=2))
                wqk_sb.append(wt)
            bv_sb = cp.tile([P, C], F32, tag="bv", name="bv")
            nc.sync.dma_start(out=bv_sb[:], in_=b_v[:])
            for kt in range(T2 // P):
                nc.gpsimd.memset(
                    v_sb[kt].rearrange("p (h d) -> p h d", d=65)[:, :, 64:65], 1.0)
            if apply_lnwb:
                w1 = cp.tile([P, C], BF16, tag="w1", name="w1")
                nc.sync.dma_start(out=w1[:], in_=ln1w[:])
                b1 = cp.tile([P, C], BF16, tag="b1", name="b1")
                nc.sync.dma_start(out=b1[:], in_=ln1b[:])
            # interleave x_q tiles early so Q blk0/blk1 (and with them the
            # first attention slots) are ready mid-phase
            ti_order = ([0, 1, 2, 3] + [16, 17, 18, 19] + [4, 5, 6, 7]
                        + [20, 21, 22, 23] + list(range(8, 16)))
            for ti in ti_order:
                is_seq = ti < 16
                tt = ti if is_seq else ti - 16
                src = x_seq if is_seq else x_q
                dstT = xlnT_big if is_seq else xlnTq_big
                xt = sp.tile([P, C], F32, tag="xs", name="xs")
                nc.sync.dma_start(out=xt[:], in_=src[tt * P:(tt + 1) * P, :])
                stats = st.tile([P, 2, 6], F32, tag="st", name="st")
                for g in range(2):
                    nc.vector.bn_stats(out=stats[:, g, :], in_=xt[:, g * 512:(g + 1) * 512])
                mv = st.tile([P, 2], F32, tag="mv", name="mv")
                nc.vector.bn_aggr(out=mv[:], in_=stats[:])
                rstd = st.tile([P, 1], F32, tag="rstd", name="rstd")
                nc.scalar.activation(rstd[:], mv[:, 1:2], AF.Sqrt, bias=eps_sb[:], scale=1.0)
                nc.vector.reciprocal(out=rstd[:], in_=rstd[:])
                xb = sp.tile([P, C], BF16, tag="xb", name="xb")
                nmr = st.tile([P, 1], F32, tag="nmr", name="nmr")
                nc.vector.tensor_scalar(nmr[:], mv[:, 0:1], rstd[:], -1.0,
                                        OP.mult, OP.mult)
                if apply_lnwb:
                    xc = sp.tile([P, C], F32, tag="xc", name="xc")
                    nc.scalar.activation(xc[:], xt[:], AF.Identity,
                                         bias=nmr[:], scale=rstd[:])
                    nc.vector.tensor_tensor(out=xc[:], in0=xc[:], in1=w1[:], op=OP.mult)
                    nc.vector.tensor_tensor(out=xb[:], in0=xc[:], in1=b1[:], op=OP.add)
                else:
                    nc.scalar.activation(xb[:], xt[:], AF.Identity,
                                         bias=nmr[:], scale=rstd[:])
                pst = tps.tile([P, CC, P], BF16, tag="trp", name="trp")
                for cc in range(CC):
                    nc.tensor.transpose(pst[:, cc, :],
                                        xb[:, cc * P:(cc + 1) * P], ident[:])
                nc.vector.tensor_copy(
                    out=dstT[:, :, tt * P:(tt + 1) * P], in_=pst[:])
                if is_seq:
                    # V for this token tile
                    for vg in range(2):
                        ps = vps.tile([P, N], F32, tag="v_ps", name="v_ps")
                        for g in range(4):
                            nc.tensor.matmul(ps[:],
                                             lhsT=xlnT_big[:, 2 * g:2 * g + 2,
                                                           tt * P:(tt + 1) * P],
                                             rhs=wv_sb[g][:, :, vg * N:(vg + 1) * N],
                                             start=(g == 0), stop=(g == 3),
                                             perf_mode=DR)
                        out_ap = v_sb[tt].rearrange("p (h d) -> p h d", d=65)[
                            :, vg * 8:(vg + 1) * 8, 0:64]
                        in_ap = ps.rearrange("p (h d) -> p h d", d=64)[:, :, :]
                        bv_ap = bv_sb.rearrange("p (h d) -> p h d", d=64)[
                            :, vg * 8:(vg + 1) * 8, :]
                        nc.vector.scalar_tensor_tensor(
                            out=out_ap, in0=in_ap, scalar=1.0 / FC_SCALE,
                            in1=bv_ap, op0=OP.mult, op1=OP.add)
                if tt % 4 == 3:
                    blk = tt // 4
                    for fc in range(CC):
                        ps = qps.tile([P, N], F32, tag="qk_ps", name="qk_ps")
                        srcT = xlnT_big if is_seq else xlnTq_big
                        fco = CC + fc if is_seq else fc
                        for g in range(4):
                            nc.tensor.matmul(ps[:],
                                             lhsT=wqk_sb[g][:, :, fco * P:(fco + 1) * P],
                                             rhs=srcT[:, 2 * g:2 * g + 2,
                                                      blk * N:(blk + 1) * N],
                                             start=(g == 0), stop=(g == 3),
                                             perf_mode=DR)
                        if is_seq:
                            nc.scalar.activation(
                                k_sb[fc][:, blk * N:(blk + 1) * N], ps[:],
                                AF.Identity, bias=bk_sb[:, fc:fc + 1],
                                scale=1.0 / FC_SCALE)
                        else:
                            nc.scalar.activation(
                                q_sb[fc][:, blk * N:(blk + 1) * N], ps[:],
                                AF.Identity, bias=bq_sb[:, fc:fc + 1],
                                scale=1.0 / FC_SCALE)

        es_xln.close()

        x_mid = [p_mid.tile([P, C], F32, tag=f"xm{i}", name=f"xm{i}") for i in range(TQ // P)]
        p_yb = es_yb.enter_context(tc.tile_pool(name="p_yb", bufs=1))
        y_fm = [p_yb.tile([P, TQ], BF16, tag=f"yf{i}", name=f"yf{i}") for i in range(CC)]
        s16 = p_yb.tile([16, TQ], F32, tag="s16", name="s16")
        mask_sb = p_yb.tile([P, 16, QW], BF16, tag="mask", name="mask")
        nc.sync.dma_start(out=mask_sb[:],
                          in_=masks[:].rearrange("p (m q) -> p m q", q=QW))
        p_pjw = es_pj.enter_context(tc.tile_pool(name="pj_w", bufs=1))
        wpj_sb = [p_pjw.tile([P, C], BF16, tag=f"wpj{i}", name=f"wpj{i}") for i in range(CC)]
        for i in range(CC):
            nc.sync.dma_start(out=wpj_sb[i][:], in_=w_pj[i * P:(i + 1) * P, :])
        if apply_bias:
            bpj_sb = p_pjw.tile([P, C], F32, tag="bpj", name="bpj")
            nc.sync.dma_start(out=bpj_sb[:], in_=b_pj[:])

        # ---------------- Phase 2: attention slots + proj, pipelined -------
        with tc.tile_pool(name="att_sp", bufs=4) as ap_pool, \
             tc.tile_pool(name="att_s1", bufs=2) as s1_pool, \
             tc.tile_pool(name="pj_sp", bufs=2) as pj_sp, \
             tc.tile_pool(name="att_sps", bufs=2, space="PSUM") as sps_pool, \
             tc.tile_pool(name="att_yps", bufs=2, space="PSUM") as yps_pool, \
             tc.tile_pool(name="att_rps", bufs=1, space="PSUM") as rps_pool, \
             tc.tile_pool(name="pj_ps", bufs=1, space="PSUM") as pps:
            for slot in range(4):
                n_sg = slot + 1
                qlo = slot * QW
                for h in range(H):
                    hp, ro = h // 2, (h % 2) * 64
                    yps = yps_pool.tile([65, QW], F32, tag="yps", name="yps")
                    for sg in range(n_sg):
                        sp4 = sps_pool.tile([P, 4, QW], F32, tag="sps", name="sps")
                        for j in range(4):
                            kt = sg * 4 + j
                            nc.tensor.matmul(sp4[:, j, :],
                                             lhsT=k_sb[hp][ro:ro + 64, kt * P:(kt + 1) * P],
                                             rhs=q_sb[hp][ro:ro + 64, qlo:qlo + QW],
                                             start=True, stop=True)
                        maskable = (sg == n_sg - 1)
                        at4 = ap_pool.tile([P, 4, QW], BF16, tag="at", name="at")
                        nc.scalar.activation(at4[:], sp4[:], AF.Exp,
                                             bias=0.0, scale=0.125)
                        if maskable:
                            nc.vector.tensor_tensor(
                                out=at4[:], in0=at4[:],
                                in1=mask_sb[:, 4 * slot:4 * slot + 4, :],
                                op=OP.mult)
                        for j in range(4):
                            kt = sg * 4 + j
                            nc.tensor.matmul(yps[:],
                                             lhsT=v_sb[kt][:, h * 65:(h + 1) * 65],
                                             rhs=at4[:, j, :],
                                             start=(sg == 0 and j == 0),
                                             stop=(sg == n_sg - 1 and j == 3))
                    # stash y and the softmax row-sums; normalize per-slot below
                    nc.vector.tensor_copy(out=y_fm[hp][ro:ro + 64, qlo:qlo + QW],
                                          in_=yps[0:64, :])
                    s1 = s1_pool.tile([1, QW], F32, tag="s1", name="s1")
                    nc.vector.tensor_copy(out=s1[:], in_=yps[64:65, :])
                    nc.sync.dma_start(out=s16[h:h + 1, qlo:qlo + QW], in_=s1[:])
                # normalize all heads for this slot: one batched reciprocal,
                # per-pair repack via SBUF->SBUF DMA, K=2 selector matmul
                # broadcast, multiply on gpsimd
                recip = s1_pool.tile([16, QW], F32, tag="recip", name="recip")
                nc.vector.reciprocal(out=recip[:], in_=s16[:, qlo:qlo + QW])
                recipr = s1_pool.tile([16, QW], F32R, tag="recipr", name="recipr")
                nc.vector.tensor_copy(out=recipr[:], in_=recip[:])
                for yt in range(CC):
                    rr2 = s1_pool.tile([2, QW], F32R, tag="rr2", name="rr2")
                    nc.sync.dma_start(out=rr2[:], in_=recipr[2 * yt:2 * yt + 2, :])
                    rps = rps_pool.tile([P, QW], F32, tag="rps", name="rps")
                    nc.tensor.matmul(rps[:], lhsT=sel_sb[:], rhs=rr2[:],
                                     start=True, stop=True)
                    nc.vector.tensor_tensor(out=y_fm[yt][:, qlo:qlo + QW],
                                            in0=y_fm[yt][:, qlo:qlo + QW],
                                            in1=rps[:], op=OP.mult)
                # proj + residual + LN2 for this slot's two 128-token tiles
                for t8 in (2 * slot, 2 * slot + 1):
                    xo = pj_sp.tile([P, C], F32, tag="xo", name="xo")
                    nc.sync.dma_start(out=xo[:], in_=x_q[t8 * P:(t8 + 1) * P, :])
                    for ft in range(2):
                        ps = pps.tile([P, N], F32, tag="pj_ps", name="pj_ps")
                        for yc in range(CC):
                            nc.tensor.matmul(ps[:],
                                             lhsT=y_fm[yc][:, t8 * P:(t8 + 1) * P],
                                             rhs=wpj_sb[yc][:, ft * N:(ft + 1) * N],
                                             start=(yc == 0), stop=(yc == CC - 1))
                        nc.vector.tensor_tensor(out=x_mid[t8][:, ft * N:(ft + 1) * N],
                                                in0=ps[:], in1=xo[:, ft * N:(ft + 1) * N],
                                                op=OP.add)
                    if apply_bias:
                        nc.vector.tensor_tensor(out=x_mid[t8][:], in0=x_mid[t8][:],
                                                in1=bpj_sb[:], op=OP.add)

        es_pj.close()
        es_yb.close()
        es_qkv.close()

        # ---------------- Phase 3: LN2 + transpose ------------------------
        p_h = es_h.enter_context(tc.tile_pool(name="p_h", bufs=1))
        h_big = p_h.tile([P, F // P, TQ], F8, tag="hbig", name="hbig")
        xln2T_big = p_h.tile([P, CC, TQ], F8, tag="xln2T", name="xln2T")
        with tc.tile_pool(name="ln2_sp", bufs=3) as sp, \
             tc.tile_pool(name="ln2_cp", bufs=1) as cp, \
             tc.tile_pool(name="ln2_st", bufs=6) as st, \
             tc.tile_pool(name="ln2_ps", bufs=2, space="PSUM") as tps:
            if apply_lnwb:
                w2 = cp.tile([P, C], BF16, tag="w2", name="w2")
                nc.sync.dma_start(out=w2[:], in_=ln2w[:])
                b2 = cp.tile([P, C], BF16, tag="b2", name="b2")
                nc.sync.dma_start(out=b2[:], in_=ln2b[:])
            for tt in range(TQ // P):
                xt = x_mid[tt]
                stats = st.tile([P, 2, 6], F32, tag="st2", name="st2")
                for g in range(2):
                    nc.vector.bn_stats(out=stats[:, g, :], in_=xt[:, g * 512:(g + 1) * 512])
                mv = st.tile([P, 2], F32, tag="mv2", name="mv2")
                nc.vector.bn_aggr(out=mv[:], in_=stats[:])
                rstd = st.tile([P, 1], F32, tag="rstd2", name="rstd2")
                nc.scalar.activation(rstd[:], mv[:, 1:2], AF.Sqrt, bias=eps_sb[:], scale=1.0)
                nc.vector.reciprocal(out=rstd[:], in_=rstd[:])
                xb = sp.tile([P, C], BF16, tag="xb2", name="xb2")
                nmr = st.tile([P, 1], F32, tag="nmr2", name="nmr2")
                nc.vector.tensor_scalar(nmr[:], mv[:, 0:1], rstd[:], -1.0,
                                        OP.mult, OP.mult)
                if apply_lnwb:
                    xc = sp.tile([P, C], F32, tag="xc2", name="xc2")
                    nc.scalar.activation(xc[:], xt[:], AF.Identity,
                                         bias=nmr[:], scale=rstd[:])
                    nc.vector.tensor_tensor(out=xc[:], in0=xc[:], in1=w2[:], op=OP.mult)
                    nc.vector.tensor_tensor(out=xb[:], in0=xc[:], in1=b2[:], op=OP.add)
                else:
                    nc.scalar.activation(xb[:], xt[:], AF.Identity,
                                         bias=nmr[:], scale=rstd[:])
                pst = tps.tile([P, CC, P], BF16, tag="trp2", name="trp2")
                for cc in range(CC):
                    nc.tensor.transpose(pst[:, cc, :],
                                        xb[:, cc * P:(cc + 1) * P], ident[:])
                nc.vector.tensor_copy(
                    out=xln2T_big[:, :, tt * P:(tt + 1) * P], in_=pst[:])

        # ---------------- Phase 4: FC + gelu (fp8 DoubleRow) ---------------
        with tc.tile_pool(name="fc_w", bufs=12) as wp, \
             tc.tile_pool(name="fc_ps", bufs=4, space="PSUM") as fps:
            for hg in range(F // N):
                wts = []
                for g in range(4):
                    wt = wp.tile([P, 2, N], F8, tag="wfc", name="wfc")
                    nc.sync.dma_start(
                        out=wt[:],
                        in_=w_fc[g * P:(g + 1) * P,
                                 hg * 2 * N:(hg + 1) * 2 * N].rearrange(
                                     "p (j n) -> p j n", j=2))
                    wts.append(wt)
                for hs in range(4):
                    hf = hg * 4 + hs
                    for tt in range(2):
                        ps = fps.tile([P, N], F32, tag="fc_ps", name="fc_ps")
                        for g in range(4):
                            nc.tensor.matmul(ps[:],
                                             lhsT=wts[g][:, :, hs * P:(hs + 1) * P],
                                             rhs=xln2T_big[:, 2 * g:2 * g + 2,
                                                           tt * N:(tt + 1) * N],
                                             start=(g == 0), stop=(g == 3),
                                             perf_mode=DR)
                        nc.scalar.activation(h_big[:, hf, tt * N:(tt + 1) * N], ps[:],
                                             AF.Gelu_apprx_tanh,
                                             bias=bfc_sb[:, hf:hf + 1],
                                             scale=1.0 / FC_SCALE)

        # ---------------- Phase 5: out matmul + residual (fp8 DoubleRow) ---
        with tc.tile_pool(name="ot_w", bufs=4) as wp, \
             tc.tile_pool(name="ot_cp", bufs=1) as cp, \
             tc.tile_pool(name="ot_sp", bufs=3) as sp, \
             tc.tile_pool(name="ot_ps", bufs=8, space="PSUM") as ops_pool:
            if apply_bias:
                bot_sb = cp.tile([P, C], F32, tag="bot", name="bot")
                nc.sync.dma_start(out=bot_sb[:], in_=b_ot[:])
            for half in range(2):
                opss = [ops_pool.tile([P, N], F32, tag="ot_ps", name="ot_ps") for _ in range(8)]
                for q in range(F // (2 * P)):
                    wt = wp.tile([P, 2, C], F8, tag="wot", name="wot")
                    nc.sync.dma_start(
                        out=wt[:],
                        in_=w_ot[q * P:(q + 1) * P, :].rearrange(
                            "p (j n) -> p j n", j=2))
                    for tc4 in range(4):
                        t8 = half * 4 + tc4
                        for ft in range(2):
                            nc.tensor.matmul(opss[tc4 * 2 + ft][:],
                                             lhsT=h_big[:, 2 * q:2 * q + 2,
                                                        t8 * P:(t8 + 1) * P],
                                             rhs=wt[:, :, ft * N:(ft + 1) * N],
                                             start=(q == 0),
                                             stop=(q == F // (2 * P) - 1),
                                             perf_mode=DR)
                for tc4 in range(4):
                    t8 = half * 4 + tc4
                    ot = sp.tile([P, C], F32, tag="ot", name="ot")
                    for ft in range(2):
                        nc.vector.scalar_tensor_tensor(
                            out=ot[:, ft * N:(ft + 1) * N],
                            in0=opss[tc4 * 2 + ft][:],
                            scalar=1.0 / OT_SCALE,
                            in1=x_mid[t8][:, ft * N:(ft + 1) * N],
                            op0=OP.mult, op1=OP.add)
                    if apply_bias:
                        nc.vector.tensor_tensor(out=ot[:], in0=ot[:], in1=bot_sb[:], op=OP.add)
                    nc.sync.dma_start(out=out_d[t8 * P:(t8 + 1) * P, :], in_=ot[:])

    nc.finalize()
    return nc


def _pack_w8(W, scale):
    """Pack W [M, C] into DoubleRow DRAM layout [4*128, 2*M]:
    out[g*128 + k, j*M + m] = W[m, g*256 + j*128 + k] * scale."""
    w = np.clip(np.asarray(W, np.float32) * scale, -240, 240)
    M = w.shape[0]
    a = w.reshape(M, 4, 2, 128).transpose(1, 3, 2, 0)
    return np.ascontiguousarray(a.reshape(512, 2 * M)).astype(ml_dtypes.float8_e4m3)


def _pack_fc8(W_fc):
    w = np.clip(np.asarray(W_fc, np.float32) * FC_SCALE, -240, 240)
    a = w.reshape(8, 512, 4, 2, 128).transpose(2, 4, 0, 3, 1)
    return np.ascontiguousarray(a.reshape(512, 8192)).astype(ml_dtypes.float8_e4m3)


def _pack_ot8(W_out):
    w = np.clip(np.asarray(W_out, np.float32) * OT_SCALE, -240, 240)
    a = w.reshape(1024, 16, 2, 128).transpose(1, 3, 2, 0)
    return np.ascontiguousarray(a.reshape(2048, 2048)).astype(ml_dtypes.float8_e4m3)


def _make_masks(chunks):
    """mask[p, (4g+i)*QW + qf] = (256*chunks[g] + qf) >= (128*(4g+i) + p)"""
    m = np.zeros((P, 16, QW), np.float32)
    pp = np.arange(P)[:, None]
    qf = np.arange(QW)[None, :]
    for g in range(4):
        c = chunks[g]
        for i in range(4):
            kt = 4 * g + i
            m[:, kt, :] = (QW * c + qf) >= (P * kt + pp)
    return np.ascontiguousarray(m.reshape(P, 16 * QW)).astype(ml_dtypes.bfloat16)


def _prep_shared(inputs):
    bf = ml_dtypes.bfloat16
    W_attn = np.asarray(inputs["W_attn"], np.float32)
    shared = {
        "w_qk": _pack_w8(W_attn[: 2 * C], FC_SCALE),
        "w_v": _pack_w8(W_attn[2 * C:], FC_SCALE),
        "w_pj": np.ascontiguousarray(np.asarray(inputs["W_proj"], np.float32).T).astype(bf),
        "w_fc": _pack_fc8(inputs["W_fc"]),
        "w_ot": _pack_ot8(inputs["W_out"]),
        "ln1w": np.ascontiguousarray(np.broadcast_to(np.asarray(inputs["ln1_w"], np.float32), (P, C))).astype(bf),
        "ln1b": np.ascontiguousarray(np.broadcast_to(np.asarray(inputs["ln1_b"], np.float32), (P, C))).astype(bf),
        "ln2w": np.ascontiguousarray(np.broadcast_to(np.asarray(inputs["ln2_w"], np.float32), (P, C))).astype(bf),
        "ln2b": np.ascontiguousarray(np.broadcast_to(np.asarray(inputs["ln2_b"], np.float32), (P, C))).astype(bf),
        "b_q": np.ascontiguousarray(np.asarray(inputs["b_attn"], np.float32)[:C].reshape(CC, P).T),
        "b_k": np.ascontiguousarray(np.asarray(inputs["b_attn"], np.float32)[C:2 * C].reshape(CC, P).T),
        "b_v": np.ascontiguousarray(np.broadcast_to(np.asarray(inputs["b_attn"], np.float32)[2 * C:], (P, C))),
        "b_pj": np.ascontiguousarray(np.broadcast_to(np.asarray(inputs["b_proj"], np.float32), (P, C))),
        "b_fc": np.ascontiguousarray(np.asarray(inputs["b_fc"], np.float32).reshape(F // P, P).T),
        "b_ot": np.ascontiguousarray(np.broadcast_to(np.asarray(inputs["b_out"], np.float32), (P, C))),
    }
    sel = np.zeros((2, P), np.float32)
    sel[0, :64] = 1.0
    sel[1, 64:] = 1.0
    shared["sel2"] = sel
    return shared


def _make_in_maps(inputs):
    x = np.asarray(inputs["x"], np.float32)
    shared = _prep_shared(inputs)
    masks = {s: _make_masks(CHUNKS[s]) for s in (0, 1)}
    in_maps = []
    for c in range(8):
        b, s = c // 2, c % 2
        m = dict(shared)
        m["x_seq"] = np.ascontiguousarray(x[b])
        m["x_q"] = np.ascontiguousarray(
            np.concatenate([x[b, ch * QW:(ch + 1) * QW] for ch in CHUNKS[s]], axis=0))
        m["masks"] = masks[s]
        in_maps.append(m)
    return in_maps


def _get_nc(apply_lnwb=True, apply_bias=True):
    key = ("nc", apply_lnwb, apply_bias)
    if key not in _CACHE:
        _CACHE[key] = _build_nc(apply_lnwb, apply_bias)
    return _CACHE[key]


def run_cores(inputs, profile=False):
    global last_exec_time_ns
    apply_lnwb = not (
        np.allclose(np.asarray(inputs["ln1_w"]), 1.0)
        and np.allclose(np.asarray(inputs["ln1_b"]), 0.0)
        and np.allclose(np.asarray(inputs["ln2_w"]), 1.0)
        and np.allclose(np.asarray(inputs["ln2_b"]), 0.0))
    apply_bias = not all(
        np.allclose(np.asarray(inputs[k]), 0.0)
        for k in ("b_attn", "b_proj", "b_fc", "b_out"))
    nc = _get_nc(apply_lnwb, apply_bias)
    in_maps = _make_in_maps(inputs)
    if profile:
        import concourse.bass_utils as bass_utils
        bass_utils.upload_artifacts = lambda tmpdir: "local://" + tmpdir
        try:
            from trn_agent_boot.trn_boot import _ntff_profile_via_ctypes
            import antenv.axon_hooks as hooks
            if hooks.get_axon_ntff_profile_hook() is None:
                hooks.set_axon_ntff_profile_hook(
                    _ntff_profile_via_ctypes("/opt/axon/libaxon_pjrt.so"))
        except Exception:
            pass
        res = bass_utils.run_bass_kernel_spmd(nc, in_maps, list(range(8)), trace=True)
        last_exec_time_ns = res.exec_time_ns
        return res.results
    return _cached_runner(nc)(in_maps)


def _cached_runner(nc):
    key = ("runner", id(nc))
    if key in _CACHE:
        return _CACHE[key]
    import jax
    import numpy as _np
    from jax.sharding import Mesh, PartitionSpec
    from jax.experimental.shard_map import shard_map
    from concourse import bass2jax, mybir as _mybir
    bass2jax.install_neuronx_cc_hook()

    part_name = nc.partition_id_tensor.name if nc.partition_id_tensor else None
    in_names, out_names, out_avals, zero_outs = [], [], [], []
    for alloc in nc.m.functions[0].allocations:
        if not isinstance(alloc, _mybir.MemoryLocationSet):
            continue
        name = alloc.memorylocations[0].name
        if alloc.kind == "ExternalInput":
            if name != part_name:
                in_names.append(name)
        elif alloc.kind == "ExternalOutput":
            out_names.append(name)
            shape = tuple(alloc.tensor_shape)
            dtype = _mybir.dt.np(alloc.dtype)
            out_avals.append(jax.core.ShapedArray(shape, dtype))
            zero_outs.append(_np.zeros(shape, dtype))
    n_params = len(in_names)
    all_names = in_names + out_names
    if part_name is not None:
        all_names = all_names + [part_name]
    donate = tuple(range(n_params, n_params + len(out_names)))

    def _body(*args):
        operands = list(args)
        if part_name is not None:
            operands.append(bass2jax.partition_id_tensor())
        outs = bass2jax._bass_exec_p.bind(
            *operands, out_avals=tuple(out_avals), in_names=tuple(all_names),
            out_names=tuple(out_names), lowering_input_output_aliases=(),
            sim_require_finite=True, sim_require_nnan=True, nc=nc)
        return tuple(outs)

    devices = jax.devices()[:8]
    mesh = Mesh(_np.asarray(devices), ("core",))
    spec = (PartitionSpec("core"),) * (n_params + len(out_names))
    sharded = jax.jit(
        shard_map(_body, mesh=mesh, in_specs=spec,
                  out_specs=(PartitionSpec("core"),) * len(out_names),
                  check_rep=False),
        donate_argnums=donate, keep_unused=True)

    def run(in_maps):
        concat_in = [
            _np.concatenate([_np.asarray(in_maps[c][nm]) for c in range(8)], axis=0)
            for nm in in_names]
        concat_zero = [_np.zeros((8 * z.shape[0], *z.shape[1:]), z.dtype)
                       for z in zero_outs]
        out_arrs = sharded(*concat_in, *concat_zero)
        return [
            {nm: _np.asarray(out_arrs[i]).reshape(8, *out_avals[i].shape)[c]
             for i, nm in enumerate(out_names)}
            for c in range(8)]

    _CACHE[key] = run
    return run


def kernel(**inputs) -> np.ndarray:
    results = run_cores(inputs, profile=PROFILE)
    out = np.empty((B, T, C), np.float32)
    for c in range(8):
        b, s = c // 2, c % 2
        r = results[c]["out"]
        for g, ch in enumerate(CHUNKS[s]):
            out[b, ch * QW:(ch + 1) * QW, :] = r[g * QW:(g + 1) * QW]
    return out


# revision 48
# speedup vs baseline: 1.2950x; 1.0629x over previous
"""Trainium2 Bass kernel for a GPT-2 style transformer block.

Problem: B=4, T=2048, C=1024, H=16 heads (hd=64), MLP hidden 4096, fp32 I/O.

Sharding: zero-collective 8-way data parallel with a *folded* sequence
split. Core c handles batch b=c//2; the pair of cores for a batch element
split the 2048 query tokens into eight 256-token chunks, interleaved so
causal attention work is balanced AND the union tile-set is SPMD-uniform:
  core s=0: chunks {0, 3, 4, 7}   core s=1: chunks {1, 2, 5, 6}
Attention runs in 4 slots; slot g covers the core's g-th chunk (ascending
need) against the natural-order key prefix of K_g = 4(g+1) key tiles of
128. The last 4 key tiles of each slot carry per-core element masks
(ones / partial-diagonal / zero) from DRAM; earlier tiles are full for
both cores. No gating, no wasted "other half" tiles.

K/V are computed for the full 2048 natural-order tokens (x_seq); Q and the
residual stream use a second input x_q holding the core's own 1024 tokens
in slot order (host-side gather).

exp() is split between the Scalar engine (true exp) and the Vector engine
(Schraudolph int-bitcast exp2 approximation) so neither is a bottleneck.

fp8 (e4m3, DoubleRow) everywhere error budget allows: QKV projections,
MLP c_fc and c_proj. Attention scores/AV and attn.c_proj stay bf16.
Weights are host-prescaled into fp8 range; the inverse scale rides the
existing bias-add / activation instructions.
"""

import os
import sys
import types
import math

import numpy as np
import ml_dtypes

for _p in ("/opt/trn_rl_repo", "/root/.axon_site/_ro/trn_rl_repo"):
    if os.path.isdir(_p) and _p not in sys.path:
        sys.path.append(_p)

if "antenv.axon_hooks" not in sys.modules:
    try:
        import antenv

        _m = types.ModuleType("antenv.axon_hooks")
        _m._hook = None

        def _set_hook(h):
            _m._hook = h

        def _get_hook():
            return _m._hook

        _m.set_axon_ntff_profile_hook = _set_hook
        _m.get_axon_ntff_profile_hook = _get_hook
        sys.modules["antenv.axon_hooks"] = _m
        antenv.axon_hooks = _m
    except ImportError:
        pass

import concourse.bacc as bacc
import concourse.tile as tile
from concourse import mybir
from concourse.masks import make_identity

P = 128
B, T, C = 4, 2048, 1024
H, HD = 16, 64
F = 4096
T2 = T
TQ = T // 2
CC = C // P
N = 512
QW = 256  # query chunk width

CHUNKS = {0: [0, 3, 4, 7], 1: [1, 2, 5, 6]}

F32 = mybir.dt.float32
F32R = mybir.dt.float32r
BF16 = mybir.dt.bfloat16
F8 = mybir.dt.float8e4
I32 = mybir.dt.int32
AF = mybir.ActivationFunctionType
OP = mybir.AluOpType
DR = mybir.MatmulPerfMode.DoubleRow

FC_SCALE = 32.0
OT_SCALE = 256.0
# Schraudolph exp: i = A*s + B; bitcast i (int32) as float32 ~= exp(0.125*s)
SCHR_A = (1 << 23) * (1.0 / math.log(2.0)) * 0.125
SCHR_B = (127.0 - 0.0434609) * (1 << 23) + 0.5

PROFILE = False
last_exec_time_ns = None

_CACHE = {}


def _exp_on_act(h):
    return True


def _build_nc(apply_lnwb: bool = True, apply_bias: bool = True):
    nc = bacc.Bacc("TRN2", target_bir_lowering=False, debug=False, num_devices=8)

    x_seq = nc.dram_tensor("x_seq", [T2, C], F32, kind="ExternalInput")
    x_q = nc.dram_tensor("x_q", [TQ, C], F32, kind="ExternalInput")
    w_qk = nc.dram_tensor("w_qk", [512, 2 * 2 * C], F8, kind="ExternalInput")
    w_v = nc.dram_tensor("w_v", [512, 2 * C], F8, kind="ExternalInput")
    w_pj = nc.dram_tensor("w_pj", [C, C], BF16, kind="ExternalInput")
    w_fc = nc.dram_tensor("w_fc", [512, 8192], F8, kind="ExternalInput")
    w_ot = nc.dram_tensor("w_ot", [2048, 2048], F8, kind="ExternalInput")
    ln1w = nc.dram_tensor("ln1w", [P, C], BF16, kind="ExternalInput")
    ln1b = nc.dram_tensor("ln1b", [P, C], BF16, kind="ExternalInput")
    ln2w = nc.dram_tensor("ln2w", [P, C], BF16, kind="ExternalInput")
    ln2b = nc.dram_tensor("ln2b", [P, C], BF16, kind="ExternalInput")
    b_q = nc.dram_tensor("b_q", [P, CC], F32, kind="ExternalInput")
    b_k = nc.dram_tensor("b_k", [P, CC], F32, kind="ExternalInput")
    b_v = nc.dram_tensor("b_v", [P, C], F32, kind="ExternalInput")
    b_pj = nc.dram_tensor("b_pj", [P, C], F32, kind="ExternalInput")
    b_fc = nc.dram_tensor("b_fc", [P, F // P], F32, kind="ExternalInput")
    b_ot = nc.dram_tensor("b_ot", [P, C], F32, kind="ExternalInput")
    masks = nc.dram_tensor("masks", [P, 16 * QW], BF16, kind="ExternalInput")
    sel2 = nc.dram_tensor("sel2", [2, P], F32R, kind="ExternalInput")

    out_d = nc.dram_tensor("out", [TQ, C], F32, kind="ExternalOutput")

    from contextlib import ExitStack

    with tile.TileContext(nc) as tc, ExitStack() as ctx:
        const = ctx.enter_context(tc.tile_pool(name="const", bufs=1))
        es_mid = ctx.enter_context(ExitStack())
        es_h = ctx.enter_context(ExitStack())
        es_yb = ctx.enter_context(ExitStack())
        es_qkv = ctx.enter_context(ExitStack())
        es_xln = ctx.enter_context(ExitStack())
        es_pj = ctx.enter_context(ExitStack())
        p_mid = es_mid.enter_context(tc.tile_pool(name="p_mid", bufs=1))
        p_qkv = es_qkv.enter_context(tc.tile_pool(name="p_qkv", bufs=1))
        p_xln = es_xln.enter_context(tc.tile_pool(name="p_xln", bufs=1))

        ident = const.tile([P, P], BF16, tag="ident", name="ident")
        make_identity(nc, ident)
        eps_sb = const.tile([P, 1], F32, tag="eps", name="eps")
        nc.vector.memset(eps_sb[:], 1e-5)
        sel_sb = const.tile([2, P], F32R, tag="sel", name="sel")
        nc.sync.dma_start(out=sel_sb[:], in_=sel2[:])
        bq_sb = const.tile([P, CC], F32, tag="bq", name="bq")
        nc.sync.dma_start(out=bq_sb[:], in_=b_q[:])
        bk_sb = const.tile([P, CC], F32, tag="bk", name="bk")
        nc.sync.dma_start(out=bk_sb[:], in_=b_k[:])
        bfc_sb = const.tile([P, F // P], F32, tag="bfc", name="bfc")
        nc.sync.dma_start(out=bfc_sb[:], in_=b_fc[:])

        q_sb = [p_qkv.tile([P, TQ], BF16, tag=f"q{i}", name=f"q{i}") for i in range(CC)]
        k_sb = [p_qkv.tile([P, T2], BF16, tag=f"k{i}", name=f"k{i}") for i in range(CC)]
        v_sb = [p_qkv.tile([P, H * 65], BF16, tag=f"v{i}", name=f"v{i}") for i in range(T2 // P)]
        xlnT_big = p_xln.tile([P, CC, T2], F8, tag="xlnT", name="xlnT")
        xlnTq_big = p_xln.tile([P, CC, TQ], F8, tag="xlnTq", name="xlnTq")

        # ---------------- Phase 1: LN1 + transposes + QKV (fp8 DoubleRow) --
        # x_seq (natural order, 16 tiles) feeds K and V; x_q (own tokens in
        # slot order, 8 tiles) feeds Q. V per tile keeps the PE busy during
        # the DVE-bound LN pipeline; K/Q blocks fire every 4th tile.
        with tc.tile_pool(name="ln1_sp", bufs=(3 if apply_lnwb else 4)) as sp, \
             tc.tile_pool(name="ln1_cp", bufs=1) as cp, \
             tc.tile_pool(name="ln1_st", bufs=8) as st, \
             tc.tile_pool(name="qkv_w", bufs=4) as wp, \
             tc.tile_pool(name="v_w", bufs=4) as vwp, \
             tc.tile_pool(name="qkv_ps", bufs=4, space="PSUM") as qps, \
             tc.tile_pool(name="v_ps", bufs=2, space="PSUM") as vps, \
             tc.tile_pool(name="ln1_ps", bufs=2, space="PSUM") as tps:
            wqk_sb = []
            wv_sb = []
            for g in range(4):
                wt = vwp.tile([P, 2, C], F8, tag="wv8", name="wv8")
                nc.sync.dma_start(out=wt[:],
                                  in_=w_v[g * P:(g + 1) * P, :].rearrange(
                                      "p (j n) -> p j n", j=2))
                wv_sb.append(wt)
            for g in range(4):
                wt = wp.tile([P, 2, 2 * C], F8, tag="w8", name="w8")
                nc.sync.dma_start(out=wt[:],
                                  in_=w_qk[g * P:(g + 1) * P, :].rearrange(
                                      "p (j n) -> p j n", j=2))
                wqk_sb.append(wt)
            bv_sb = cp.tile([P, C], F32, tag="bv", name="bv")
            nc.sync.dma_start(out=bv_sb[:], in_=b_v[:])
            for kt in range(T2 // P):
                nc.gpsimd.memset(
                    v_sb[kt].rearrange("p (h d) -> p h d", d=65)[:, :, 64:65], 1.0)
            if apply_lnwb:
                w1 = cp.tile([P, C], BF16, tag="w1", name="w1")
                nc.sync.dma_start(out=w1[:], in_=ln1w[:])
                b1 = cp.tile([P, C], BF16, tag="b1", name="b1")
                nc.sync.dma_start(out=b1[:], in_=ln1b[:])
            # interleave x_q tiles early so Q blk0/blk1 (and with them the
            # first attention slots) are ready mid-phase
            ti_order = ([0, 1, 2, 3] + [16, 17, 18, 19] + [4, 5, 6, 7]
                        + [20, 21, 22, 23] + list(range(8, 16)))
            for ti in ti_order:
                is_seq = ti < 16
                tt = ti if is_seq else ti - 16
                src = x_seq if is_seq else x_q
                dstT = xlnT_big if is_seq else xlnTq_big
                xt = sp.tile([P, C], F32, tag="xs", name="xs")
                stats = st.tile([P, 2, 6], F32, tag="st", name="st")
                for g in range(2):
                    nc.sync.dma_start(out=xt[:, g * 512:(g + 1) * 512],
                                      in_=src[tt * P:(tt + 1) * P, g * 512:(g + 1) * 512])
                for g in range(2):
                    nc.vector.bn_stats(out=stats[:, g, :], in_=xt[:, g * 512:(g + 1) * 512])
                mv = st.tile([P, 2], F32, tag="mv", name="mv")
                nc.vector.bn_aggr(out=mv[:], in_=stats[:])
                rstd = st.tile([P, 1], F32, tag="rstd", name="rstd")
                nc.scalar.activation(rstd[:], mv[:, 1:2], AF.Sqrt, bias=eps_sb[:], scale=1.0)
                nc.vector.reciprocal(out=rstd[:], in_=rstd[:])
                xb = sp.tile([P, C], BF16, tag="xb", name="xb")
                nmr = st.tile([P, 1], F32, tag="nmr", name="nmr")
                nc.vector.tensor_scalar(nmr[:], mv[:, 0:1], rstd[:], -1.0,
                                        OP.mult, OP.mult)
                if apply_lnwb:
                    xc = sp.tile([P, C], F32, tag="xc", name="xc")
                    nc.scalar.activation(xc[:], xt[:], AF.Identity,
                                         bias=nmr[:], scale=rstd[:])
                    nc.vector.tensor_tensor(out=xc[:], in0=xc[:], in1=w1[:], op=OP.mult)
                    nc.vector.tensor_tensor(out=xb[:], in0=xc[:], in1=b1[:], op=OP.add)
                else:
                    nc.scalar.activation(xb[:], xt[:], AF.Identity,
                                         bias=nmr[:], scale=rstd[:])
                pst = tps.tile([P, CC, P], BF16, tag="trp", name="trp")
                for cc in range(CC):
                    nc.tensor.transpose(pst[:, cc, :],
                                        xb[:, cc * P:(cc + 1) * P], ident[:])
                nc.vector.tensor_copy(
                    out=dstT[:, :, tt * P:(tt + 1) * P], in_=pst[:])
                if is_seq:
                    # V for this token tile
                    for vg in range(2):
                        ps = vps.tile([P, N], F32, tag="v_ps", name="v_ps")
                        for g in range(4):
                            nc.tensor.matmul(ps[:],
                                             lhsT=xlnT_big[:, 2 * g:2 * g + 2,
                                                           tt * P:(tt + 1) * P],
                                             rhs=wv_sb[g][:, :, vg * N:(vg + 1) * N],
                                             start=(g == 0), stop=(g == 3),
                                             perf_mode=DR)
                        out_ap = v_sb[tt].rearrange("p (h d) -> p h d", d=65)[
                            :, vg * 8:(vg + 1) * 8, 0:64]
                        in_ap = ps.rearrange("p (h d) -> p h d", d=64)[:, :, :]
                        bv_ap = bv_sb.rearrange("p (h d) -> p h d", d=64)[
                            :, vg * 8:(vg + 1) * 8, :]
                        nc.vector.scalar_tensor_tensor(
                            out=out_ap, in0=in_ap, scalar=1.0 / FC_SCALE,
                            in1=bv_ap, op0=OP.mult, op1=OP.add)
                if tt % 4 == 3:
                    blk = tt // 4
                    for fc in range(CC):
                        ps = qps.tile([P, N], F32, tag="qk_ps", name="qk_ps")
                        srcT = xlnT_big if is_seq else xlnTq_big
                        fco = CC + fc if is_seq else fc
                        for g in range(4):
                            nc.tensor.matmul(ps[:],
                                             lhsT=wqk_sb[g][:, :, fco * P:(fco + 1) * P],
                                             rhs=srcT[:, 2 * g:2 * g + 2,
                                                      blk * N:(blk + 1) * N],
                                             start=(g == 0), stop=(g == 3),
                                             perf_mode=DR)
                        if is_seq:
                            nc.scalar.activation(
                                k_sb[fc][:, blk * N:(blk + 1) * N], ps[:],
                                AF.Identity, bias=bk_sb[:, fc:fc + 1],
                                scale=1.0 / FC_SCALE)
                        else:
                            nc.scalar.activation(
                                q_sb[fc][:, blk * N:(blk + 1) * N], ps[:],
                                AF.Identity, bias=bq_sb[:, fc:fc + 1],
                                scale=1.0 / FC_SCALE)

        es_xln.close()

        x_mid = [p_mid.tile([P, C], F32, tag=f"xm{i}", name=f"xm{i}") for i in range(TQ // P)]
        p_yb = es_yb.enter_context(tc.tile_pool(name="p_yb", bufs=1))
        y_fm = [p_yb.tile([P, TQ], BF16, tag=f"yf{i}", name=f"yf{i}") for i in range(CC)]
        s16 = p_yb.tile([16, TQ], F32, tag="s16", name="s16")
        mask_sb = p_yb.tile([P, 16, QW], BF16, tag="mask", name="mask")
        nc.sync.dma_start(out=mask_sb[:],
                          in_=masks[:].rearrange("p (m q) -> p m q", q=QW))
        p_pjw = es_pj.enter_context(tc.tile_pool(name="pj_w", bufs=1))
        wpj_sb = [p_pjw.tile([P, C], BF16, tag=f"wpj{i}", name=f"wpj{i}") for i in range(CC)]
        for i in range(CC):
            nc.sync.dma_start(out=wpj_sb[i][:], in_=w_pj[i * P:(i + 1) * P, :])
        if apply_bias:
            bpj_sb = p_pjw.tile([P, C], F32, tag="bpj", name="bpj")
            nc.sync.dma_start(out=bpj_sb[:], in_=b_pj[:])

        # ---------------- Phase 2: attention slots + proj, pipelined -------
        with tc.tile_pool(name="att_sp", bufs=4) as ap_pool, \
             tc.tile_pool(name="att_s1", bufs=2) as s1_pool, \
             tc.tile_pool(name="pj_sp", bufs=2) as pj_sp, \
             tc.tile_pool(name="att_sps", bufs=2, space="PSUM") as sps_pool, \
             tc.tile_pool(name="att_yps", bufs=2, space="PSUM") as yps_pool, \
             tc.tile_pool(name="att_rps", bufs=1, space="PSUM") as rps_pool, \
             tc.tile_pool(name="pj_ps", bufs=1, space="PSUM") as pps:
            for slot in range(4):
                n_sg = slot + 1
                qlo = slot * QW
                for h in range(H):
                    hp, ro = h // 2, (h % 2) * 64
                    yps = yps_pool.tile([65, QW], F32, tag="yps", name="yps")
                    for sg in range(n_sg):
                        sp4 = sps_pool.tile([P, 4, QW], F32, tag="sps", name="sps")
                        for j in range(4):
                            kt = sg * 4 + j
                            nc.tensor.matmul(sp4[:, j, :],
                                             lhsT=k_sb[hp][ro:ro + 64, kt * P:(kt + 1) * P],
                                             rhs=q_sb[hp][ro:ro + 64, qlo:qlo + QW],
                                             start=True, stop=True)
                        maskable = (sg == n_sg - 1)
                        at4 = ap_pool.tile([P, 4, QW], BF16, tag="at", name="at")
                        nc.scalar.activation(at4[:], sp4[:], AF.Exp,
                                             bias=0.0, scale=0.125)
                        if maskable:
                            nc.vector.tensor_tensor(
                                out=at4[:], in0=at4[:],
                                in1=mask_sb[:, 4 * slot:4 * slot + 4, :],
                                op=OP.mult)
                        for j in range(4):
                            kt = sg * 4 + j
                            nc.tensor.matmul(yps[:],
                                             lhsT=v_sb[kt][:, h * 65:(h + 1) * 65],
                                             rhs=at4[:, j, :],
                                             start=(sg == 0 and j == 0),
                                             stop=(sg == n_sg - 1 and j == 3))
                    # stash y and the softmax row-sums; normalize per-slot below
                    nc.vector.tensor_copy(out=y_fm[hp][ro:ro + 64, qlo:qlo + QW],
                                          in_=yps[0:64, :])
                    s1 = s1_pool.tile([1, QW], F32, tag="s1", name="s1")
                    nc.vector.tensor_copy(out=s1[:], in_=yps[64:65, :])
                    nc.sync.dma_start(out=s16[h:h + 1, qlo:qlo + QW], in_=s1[:])
                # normalize all heads for this slot: one batched reciprocal,
                # per-pair repack via SBUF->SBUF DMA, K=2 selector matmul
                # broadcast, multiply on gpsimd
                recip = s1_pool.tile([16, QW], F32, tag="recip", name="recip")
                nc.vector.reciprocal(out=recip[:], in_=s16[:, qlo:qlo + QW])
                recipr = s1_pool.tile([16, QW], F32R, tag="recipr", name="recipr")
                nc.vector.tensor_copy(out=recipr[:], in_=recip[:])
                for yt in range(CC):
                    rr2 = s1_pool.tile([2, QW], F32R, tag="rr2", name="rr2")
                    nc.sync.dma_start(out=rr2[:], in_=recipr[2 * yt:2 * yt + 2, :])
                    rps = rps_pool.tile([P, QW], F32, tag="rps", name="rps")
                    nc.tensor.matmul(rps[:], lhsT=sel_sb[:], rhs=rr2[:],
                                     start=True, stop=True)
                    nc.vector.tensor_tensor(out=y_fm[yt][:, qlo:qlo + QW],
                                            in0=y_fm[yt][:, qlo:qlo + QW],
                                            in1=rps[:], op=OP.mult)
                # proj + residual + LN2 for this slot's two 128-token tiles
                for t8 in (2 * slot, 2 * slot + 1):
                    xo = pj_sp.tile([P, C], F32, tag="xo", name="xo")
                    nc.sync.dma_start(out=xo[:], in_=x_q[t8 * P:(t8 + 1) * P, :])
                    for ft in range(2):
                        ps = pps.tile([P, N], F32, tag="pj_ps", name="pj_ps")
                        for yc in range(CC):
                            nc.tensor.matmul(ps[:],
                                             lhsT=y_fm[yc][:, t8 * P:(t8 + 1) * P],
                                             rhs=wpj_sb[yc][:, ft * N:(ft + 1) * N],
                                             start=(yc == 0), stop=(yc == CC - 1))
                        nc.vector.tensor_tensor(out=x_mid[t8][:, ft * N:(ft + 1) * N],
                                                in0=ps[:], in1=xo[:, ft * N:(ft + 1) * N],
                                                op=OP.add)
                    if apply_bias:
                        nc.vector.tensor_tensor(out=x_mid[t8][:], in0=x_mid[t8][:],
                                                in1=bpj_sb[:], op=OP.add)

        es_pj.close()
        es_yb.close()
        es_qkv.close()

        # ---------------- Phase 3: LN2 + transpose ------------------------
        p_h = es_h.enter_context(tc.tile_pool(name="p_h", bufs=1))
        h_big = p_h.tile([P, F // P, TQ], F8, tag="hbig", name="hbig")
        xln2T_big = p_h.tile([P, CC, TQ], F8, tag="xln2T", name="xln2T")
        with tc.tile_pool(name="ln2_sp", bufs=3) as sp, \
             tc.tile_pool(name="ln2_cp", bufs=1) as cp, \
             tc.tile_pool(name="ln2_st", bufs=6) as st, \
             tc.tile_pool(name="ln2_ps", bufs=2, space="PSUM") as tps:
            if apply_lnwb:
                w2 = cp.tile([P, C], BF16, tag="w2", name="w2")
                nc.sync.dma_start(out=w2[:], in_=ln2w[:])
                b2 = cp.tile([P, C], BF16, tag="b2", name="b2")
                nc.sync.dma_start(out=b2[:], in_=ln2b[:])
            for tt in range(TQ // P):
                xt = x_mid[tt]
                stats = st.tile([P, 2, 6], F32, tag="st2", name="st2")
                for g in range(2):
                    nc.vector.bn_stats(out=stats[:, g, :], in_=xt[:, g * 512:(g + 1) * 512])
                mv = st.tile([P, 2], F32, tag="mv2", name="mv2")
                nc.vector.bn_aggr(out=mv[:], in_=stats[:])
                rstd = st.tile([P, 1], F32, tag="rstd2", name="rstd2")
                nc.scalar.activation(rstd[:], mv[:, 1:2], AF.Sqrt, bias=eps_sb[:], scale=1.0)
                nc.vector.reciprocal(out=rstd[:], in_=rstd[:])
                xb = sp.tile([P, C], BF16, tag="xb2", name="xb2")
                nmr = st.tile([P, 1], F32, tag="nmr2", name="nmr2")
                nc.vector.tensor_scalar(nmr[:], mv[:, 0:1], rstd[:], -1.0,
                                        OP.mult, OP.mult)
                if apply_lnwb:
                    xc = sp.tile([P, C], F32, tag="xc2", name="xc2")
                    nc.scalar.activation(xc[:], xt[:], AF.Identity,
                                         bias=nmr[:], scale=rstd[:])
                    nc.vector.tensor_tensor(out=xc[:], in0=xc[:], in1=w2[:], op=OP.mult)
                    nc.vector.tensor_tensor(out=xb[:], in0=xc[:], in1=b2[:], op=OP.add)
                else:
                    nc.scalar.activation(xb[:], xt[:], AF.Identity,
                                         bias=nmr[:], scale=rstd[:])
                pst = tps.tile([P, CC, P], BF16, tag="trp2", name="trp2")
                for cc in range(CC):
                    nc.tensor.transpose(pst[:, cc, :],
                                        xb[:, cc * P:(cc + 1) * P], ident[:])
                nc.vector.tensor_copy(
                    out=xln2T_big[:, :, tt * P:(tt + 1) * P], in_=pst[:])

        # ---------------- Phase 4: FC + gelu (fp8 DoubleRow) ---------------
        with tc.tile_pool(name="fc_w", bufs=12) as wp, \
             tc.tile_pool(name="fc_ps", bufs=4, space="PSUM") as fps:
            for hg in range(F // N):
                wts = []
                for g in range(4):
                    wt = wp.tile([P, 2, N], F8, tag="wfc", name="wfc")
                    nc.sync.dma_start(
                        out=wt[:],
                        in_=w_fc[g * P:(g + 1) * P,
                                 hg * 2 * N:(hg + 1) * 2 * N].rearrange(
                                     "p (j n) -> p j n", j=2))
                    wts.append(wt)
                for hs in range(4):
                    hf = hg * 4 + hs
                    for tt in range(2):
                        ps = fps.tile([P, N], F32, tag="fc_ps", name="fc_ps")
                        for g in range(4):
                            nc.tensor.matmul(ps[:],
                                             lhsT=wts[g][:, :, hs * P:(hs + 1) * P],
                                             rhs=xln2T_big[:, 2 * g:2 * g + 2,
                                                           tt * N:(tt + 1) * N],
                                             start=(g == 0), stop=(g == 3),
                                             perf_mode=DR)
                        nc.scalar.activation(h_big[:, hf, tt * N:(tt + 1) * N], ps[:],
                                             AF.Gelu_apprx_tanh,
                                             bias=bfc_sb[:, hf:hf + 1],
                                             scale=1.0 / FC_SCALE)

        # ---------------- Phase 5: out matmul + residual (fp8 DoubleRow) ---
        with tc.tile_pool(name="ot_w", bufs=4) as wp, \
             tc.tile_pool(name="ot_cp", bufs=1) as cp, \
             tc.tile_pool(name="ot_sp", bufs=3) as sp, \
             tc.tile_pool(name="ot_ps", bufs=8, space="PSUM") as ops_pool:
            if apply_bias:
                bot_sb = cp.tile([P, C], F32, tag="bot", name="bot")
                nc.sync.dma_start(out=bot_sb[:], in_=b_ot[:])
            for half in range(2):
                opss = [ops_pool.tile([P, N], F32, tag="ot_ps", name="ot_ps") for _ in range(8)]
                for q in range(F // (2 * P)):
                    wt = wp.tile([P, 2, C], F8, tag="wot", name="wot")
                    nc.sync.dma_start(
                        out=wt[:],
                        in_=w_ot[q * P:(q + 1) * P, :].rearrange(
                            "p (j n) -> p j n", j=2))
                    for tc4 in range(4):
                        t8 = half * 4 + tc4
                        for ft in range(2):
                            nc.tensor.matmul(opss[tc4 * 2 + ft][:],
                                             lhsT=h_big[:, 2 * q:2 * q + 2,
                                                        t8 * P:(t8 + 1) * P],
                                             rhs=wt[:, :, ft * N:(ft + 1) * N],
                                             start=(q == 0),
                                             stop=(q == F // (2 * P) - 1),
                                             perf_mode=DR)
                for tc4 in range(4):
                    t8 = half * 4 + tc4
                    ot = sp.tile([P, C], F32, tag="ot", name="ot")
                    for ft in range(2):
                        nc.vector.scalar_tensor_tensor(
                            out=ot[:, ft * N:(ft + 1) * N],
                            in0=opss[tc4 * 2 + ft][:],
                            scalar=1.0 / OT_SCALE,
                            in1=x_mid[t8][:, ft * N:(ft + 1) * N],
                            op0=OP.mult, op1=OP.add)
                    if apply_bias:
                        nc.vector.tensor_tensor(out=ot[:], in0=ot[:], in1=bot_sb[:], op=OP.add)
                    nc.sync.dma_start(out=out_d[t8 * P:(t8 + 1) * P, :], in_=ot[:])

    nc.finalize()
    return nc


def _pack_w8(W, scale):
    """Pack W [M, C] into DoubleRow DRAM layout [4*128, 2*M]:
    out[g*128 + k, j*M + m] = W[m, g*256 + j*128 + k] * scale."""
    w = np.clip(np.asarray(W, np.float32) * scale, -240, 240)
    M = w.shape[0]
    a = w.reshape(M, 4, 2, 128).transpose(1, 3, 2, 0)
    return np.ascontiguousarray(a.reshape(512, 2 * M)).astype(ml_dtypes.float8_e4m3)


def _pack_fc8(W_fc):
    w = np.clip(np.asarray(W_fc, np.float32) * FC_SCALE, -240, 240)
    a = w.reshape(8, 512, 4, 2, 128).transpose(2, 4, 0, 3, 1)
    return np.ascontiguousarray(a.reshape(512, 8192)).astype(ml_dtypes.float8_e4m3)


def _pack_ot8(W_out):
    w = np.clip(np.asarray(W_out, np.float32) * OT_SCALE, -240, 240)
    a = w.reshape(1024, 16, 2, 128).transpose(1, 3, 2, 0)
    return np.ascontiguousarray(a.reshape(2048, 2048)).astype(ml_dtypes.float8_e4m3)


def _make_masks(chunks):
    """mask[p, (4g+i)*QW + qf] = (256*chunks[g] + qf) >= (128*(4g+i) + p)"""
    m = np.zeros((P, 16, QW), np.float32)
    pp = np.arange(P)[:, None]
    qf = np.arange(QW)[None, :]
    for g in range(4):
        c = chunks[g]
        for i in range(4):
            kt = 4 * g + i
            m[:, kt, :] = (QW * c + qf) >= (P * kt + pp)
    return np.ascontiguousarray(m.reshape(P, 16 * QW)).astype(ml_dtypes.bfloat16)


def _prep_shared(inputs):
    bf = ml_dtypes.bfloat16
    W_attn = np.asarray(inputs["W_attn"], np.float32)
    shared = {
        "w_qk": _pack_w8(W_attn[: 2 * C], FC_SCALE),
        "w_v": _pack_w8(W_attn[2 * C:], FC_SCALE),
        "w_pj": np.ascontiguousarray(np.asarray(inputs["W_proj"], np.float32).T).astype(bf),
        "w_fc": _pack_fc8(inputs["W_fc"]),
        "w_ot": _pack_ot8(inputs["W_out"]),
        "ln1w": np.ascontiguousarray(np.broadcast_to(np.asarray(inputs["ln1_w"], np.float32), (P, C))).astype(bf),
        "ln1b": np.ascontiguousarray(np.broadcast_to(np.asarray(inputs["ln1_b"], np.float32), (P, C))).astype(bf),
        "ln2w": np.ascontiguousarray(np.broadcast_to(np.asarray(inputs["ln2_w"], np.float32), (P, C))).astype(bf),
        "ln2b": np.ascontiguousarray(np.broadcast_to(np.asarray(inputs["ln2_b"], np.float32), (P, C))).astype(bf),
        "b_q": np.ascontiguousarray(np.asarray(inputs["b_attn"], np.float32)[:C].reshape(CC, P).T),
        "b_k": np.ascontiguousarray(np.asarray(inputs["b_attn"], np.float32)[C:2 * C].reshape(CC, P).T),
        "b_v": np.ascontiguousarray(np.broadcast_to(np.asarray(inputs["b_attn"], np.float32)[2 * C:], (P, C))),
        "b_pj": np.ascontiguousarray(np.broadcast_to(np.asarray(inputs["b_proj"], np.float32), (P, C))),
        "b_fc": np.ascontiguousarray(np.asarray(inputs["b_fc"], np.float32).reshape(F // P, P).T),
        "b_ot": np.ascontiguousarray(np.broadcast_to(np.asarray(inputs["b_out"], np.float32), (P, C))),
    }
    sel = np.zeros((2, P), np.float32)
    sel[0, :64] = 1.0
    sel[1, 64:] = 1.0
    shared["sel2"] = sel
    return shared


def _make_in_maps(inputs):
    x = np.asarray(inputs["x"], np.float32)
    shared = _prep_shared(inputs)
    masks = {s: _make_masks(CHUNKS[s]) for s in (0, 1)}
    in_maps = []
    for c in range(8):
        b, s = c // 2, c % 2
        m = dict(shared)
        m["x_seq"] = np.ascontiguousarray(x[b])
        m["x_q"] = np.ascontiguousarray(
            np.concatenate([x[b, ch * QW:(ch + 1) * QW] for ch in CHUNKS[s]], axis=0))
        m["masks"] = masks[s]
        in_maps.append(m)
    return in_maps


def _get_nc(apply_lnwb=True, apply_bias=True):
    key = ("nc", apply_lnwb, apply_bias)
    if key not in _CACHE:
        _CACHE[key] = _build_nc(apply_lnwb, apply_bias)
    return _CACHE[key]


def run_cores(inputs, profile=False):
    global last_exec_time_ns
    apply_lnwb = not (
        np.allclose(np.asarray(inputs["ln1_w"]), 1.0)
        and np.allclose(np.asarray(inputs["ln1_b"]), 0.0)
        and np.allclose(np.asarray(inputs["ln2_w"]), 1.0)
        and np.allclose(np.asarray(inputs["ln2_b"]), 0.0))
    apply_bias = not all(
        np.allclose(np.asarray(inputs[k]), 0.0)
        for k in ("b_attn", "b_proj", "b_fc", "b_out"))
    nc = _get_nc(apply_lnwb, apply_bias)
    in_maps = _make_in_maps(inputs)
    if profile:
        import concourse.bass_utils as bass_utils
        bass_utils.upload_artifacts = lambda tmpdir: "local://" + tmpdir
        try:
            from trn_agent_boot.trn_boot import _ntff_profile_via_ctypes
            import antenv.axon_hooks as hooks
            if hooks.get_axon_ntff_profile_hook() is None:
                hooks.set_axon_ntff_profile_hook(
                    _ntff_profile_via_ctypes("/opt/axon/libaxon_pjrt.so"))
        except Exception:
            pass
        res = bass_utils.run_bass_kernel_spmd(nc, in_maps, list(range(8)), trace=True)
        last_exec_time_ns = res.exec_time_ns
        return res.results
    return _cached_runner(nc)(in_maps)


def _cached_runner(nc):
    key = ("runner", id(nc))
    if key in _CACHE:
        return _CACHE[key]
    import jax
    import numpy as _np
    from jax.sharding import Mesh, PartitionSpec
    from jax.experimental.shard_map import shard_map
    from concourse import bass2jax, mybir as _mybir
    bass2jax.install_neuronx_cc_hook()

    part_name = nc.partition_id_tensor.name if nc.partition_id_tensor else None
    in_names, out_names, out_avals, zero_outs = [], [], [], []
    for alloc in nc.m.functions[0].allocations:
        if not isinstance(alloc, _mybir.MemoryLocationSet):
            continue
        name = alloc.memorylocations[0].name
        if alloc.kind == "ExternalInput":
            if name != part_name:
                in_names.append(name)
        elif alloc.kind == "ExternalOutput":
            out_names.append(name)
            shape = tuple(alloc.tensor_shape)
            dtype = _mybir.dt.np(alloc.dtype)
            out_avals.append(jax.core.ShapedArray(shape, dtype))
            zero_outs.append(_np.zeros(shape, dtype))
    n_params = len(in_names)
    all_names = in_names + out_names
    if part_name is not None:
        all_names = all_names + [part_name]
    donate = tuple(range(n_params, n_params + len(out_names)))

    def _body(*args):
        operands = list(args)
        if part_name is not None:
            operands.append(bass2jax.partition_id_tensor())
        outs = bass2jax._bass_exec_p.bind(
            *operands, out_avals=tuple(out_avals), in_names=tuple(all_names),
            out_names=tuple(out_names), lowering_input_output_aliases=(),
            sim_require_finite=True, sim_require_nnan=True, nc=nc)
        return tuple(outs)

    devices = jax.devices()[:8]
    mesh = Mesh(_np.asarray(devices), ("core",))
    spec = (PartitionSpec("core"),) * (n_params + len(out_names))
    sharded = jax.jit(
        shard_map(_body, mesh=mesh, in_specs=spec,
                  out_specs=(PartitionSpec("core"),) * len(out_names),
                  check_rep=False),
        donate_argnums=donate, keep_unused=True)

    def run(in_maps):
        concat_in = [
            _np.concatenate([_np.asarray(in_maps[c][nm]) for c in range(8)], axis=0)
            for nm in in_names]
        concat_zero = [_np.zeros((8 * z.shape[0], *z.shape[1:]), z.dtype)
                       for z in zero_outs]
        out_arrs = sharded(*concat_in, *concat_zero)
        return [
            {nm: _np.asarray(out_arrs[i]).reshape(8, *out_avals[i].shape)[c]
             for i, nm in enumerate(out_names)}
            for c in range(8)]

    _CACHE[key] = run
    return run


def kernel(**inputs) -> np.ndarray:
    results = run_cores(inputs, profile=PROFILE)
    out = np.empty((B, T, C), np.float32)
    for c in range(8):
        b, s = c // 2, c % 2
        r = results[c]["out"]
        for g, ch in enumerate(CHUNKS[s]):
            out[b, ch * QW:(ch + 1) * QW, :] = r[g * QW:(g + 1) * QW]
    return out
